# revision 19
# baseline (speedup 1.0000x reference)
"""HeteroGAT (3-relation, 2-layer GAT + linear head) on 8 Trainium2 cores.

v3: Layer 1 is gather-free: the host pre-permutes x rows into per-edge
order (src and dst streams, f-major), streamed contiguously via HWDGE;
per-chunk projection matmuls produce h|al_s and accumulate al_d into the
same PSUM columns. Layer 2 keeps per-relation h2 tables in HBM with
dma_gather (768B rows), now 3-deep buffered. One-hots are built with
full-rate tensor_scalar is_equal ops (oh) and partition-broadcast
is_equal (ohT); exp(leaky(z)) = max(exp(z), exp(0.2z)) via two ACT ops.
Scatter is a single 260-wide matmul per 128-edge chunk with the softmax
denominators in columns 256:260. Only cross-core exchange: AllGather of
transposed layer-1 activations (bf16).
"""

import numpy as np

import concourse.bacc as bacc
import concourse.bass as bass
import concourse.mybir as mybir
import concourse.tile as tile
from concourse.bass_utils import run_bass_kernel_spmd

F32 = mybir.dt.float32
BF16 = mybir.dt.bfloat16
I16 = mybir.dt.int16

N = 50000
NPAD = 50176            # 392 * 128
NCORES = 8
NOWN = 6272             # 49 * 128 rows per core
W = 49                  # window slots per core
NT = NPAD // 128        # 392 global node tiles / windows
SPLIT = 32768           # int16 index limit for dma_gather
TABW = 384              # L2 table row stride in bf16 elems (768 B, %256)
HC = 256                # feature columns
MMN = 260               # 256 feats + 4 attention cols
IN_CH = 128
HID = 64
HEADS = 4
GMAX = 8                # chunks per dma_gather call (>=2048 idx crashes)

last_results = None


# ----------------------------------------------------------------- host prep

def _bf16(a):
    import ml_dtypes
    return np.asarray(a, np.float32).astype(ml_dtypes.bfloat16)


def _assign_windows(edge_sets):
    """Snake-assign the 392 global windows to (core, slot) by total edge
    count. Returns win_of [NCORES, W] and core_of/slot_of [NT]."""
    score = np.zeros(NT, np.int64)
    for src, dst in edge_sets:
        score += np.bincount(dst >> 7, minlength=NT)
    order = np.argsort(-score, kind="stable")
    win_of = np.zeros((NCORES, W), np.int64)
    for s in range(W):
        grp = order[s * NCORES:(s + 1) * NCORES]
        if s % 2:
            grp = grp[::-1]
        for c in range(NCORES):
            win_of[c, s] = grp[c]
    core_of = np.zeros(NT, np.int64)
    slot_of = np.zeros(NT, np.int64)
    for c in range(NCORES):
        for s in range(W):
            core_of[win_of[c, s]] = c
            slot_of[win_of[c, s]] = s
    return win_of, core_of, slot_of


def _prep_edges_l1(src, dst, core_of, slot_of, xTb):
    """Layer-1 host pre-gather. Edges bucketed by (core, slot); chunk counts
    maxed over cores so the device schedule is common. Returns
    (K1 [W], per_core list of (xs [128, totch*128] bf16,
    xd [...], dl [128, totch] bf16))."""
    E = src.size
    wg = dst >> 7
    sv = slot_of[wg]
    cv = core_of[wg]
    okey = cv * W + sv
    order = np.argsort(okey, kind="stable")
    so, ss, sd = okey[order], src[order], dst[order]
    cnts = np.bincount(okey, minlength=NCORES * W)
    seg_start = np.concatenate([[0], np.cumsum(cnts)[:-1]])
    pos = np.arange(E) - seg_start[so]

    mx = cnts.reshape(NCORES, W).max(axis=0)
    K1 = np.maximum((mx + 127) // 128, 1)
    chbase = np.concatenate([[0], np.cumsum(K1)[:-1]])
    totch = int(K1.sum())

    svs, cvs = sv[order], cv[order]
    col = (chbase[svs] + (pos >> 7)) * 128 + (pos & 127)

    per_core = []
    for c in range(NCORES):
        m = cvs == c
        srccols = np.zeros(totch * 128, np.int64)
        dstcols = np.zeros(totch * 128, np.int64)
        dlv = np.full(totch * 128, -1.0, np.float32)
        srccols[col[m]] = ss[m]
        dstcols[col[m]] = sd[m]
        dlv[col[m]] = (sd[m] & 127).astype(np.float32)
        xs = np.ascontiguousarray(xTb[:, srccols])
        xd = np.ascontiguousarray(xTb[:, dstcols])
        dl = np.ascontiguousarray(dlv.reshape(totch, 128).T)
        per_core.append((xs, xd, dl))
    return dict(K1=K1, chbase=chbase, totch=totch), per_core


def _prep_edges_l2(src, dst, core_of, slot_of):
    """Layer-2 gather schedule (same as baseline) + dlT row for ohT builds.
    Returns (sched, per_core list of (idx [128, totcol] i16,
    dl [128, totch] bf16, dlT [1, totch*128] bf16))."""
    E = src.size
    wg = dst >> 7
    sv = slot_of[wg]
    cv = core_of[wg]
    gv = (src >= SPLIT).astype(np.int64)
    okey = cv * (2 * W) + sv * 2 + gv
    order = np.argsort(okey, kind="stable")
    so, ss, sd = okey[order], src[order], dst[order]
    cnts = np.bincount(okey, minlength=NCORES * 2 * W)
    seg_start = np.concatenate([[0], np.cumsum(cnts)[:-1]])
    pos = np.arange(E) - seg_start[so]

    mx = cnts.reshape(NCORES, 2 * W).max(axis=0)
    Kg = (mx + 127) // 128
    KL, KH = Kg[0::2].copy(), Kg[1::2].copy()
    KL[(KL + KH) == 0] = 1
    Ksum = KL + KH
    chbase = np.concatenate([[0], np.cumsum(Ksum)[:-1]])
    totch = int(Ksum.sum())
    colL = np.zeros(W, np.int64)
    colH = np.zeros(W, np.int64)
    cum = 0
    for w in range(W):
        colL[w] = cum
        cum += KL[w] * 8
        colH[w] = cum
        cum += KH[w] * 8
    totcol = int(cum)

    svs, gvs, cvs = sv[order], gv[order], cv[order]
    j = np.where(gvs == 0, pos, KL[svs] * 128 + pos)
    ch = chbase[svs] + (j >> 7)
    prow = j & 127
    colbase = np.where(gvs == 0, colL[svs], colH[svs])
    icol = colbase + (pos >> 4)
    irow = pos & 15
    idxval = np.where(gvs == 0, ss, ss - SPLIT).astype(np.int16)
    dloc = (sd & 127).astype(np.float32)

    eye_ext = np.zeros((129, 128), np.float32)
    eye_ext[1:] = np.eye(128, dtype=np.float32)
    eye_ext = _bf16(eye_ext)
    per_core = []
    for c in range(NCORES):
        m = cvs == c
        ia = np.zeros((16, totcol), np.int16)
        ia[irow[m], icol[m]] = idxval[m]
        dlv = np.full(totch * 128, -1, np.int64)
        dlv[ch[m] * 128 + prow[m]] = sd[m] & 127
        oh_rows = eye_ext[dlv + 1].reshape(totch, 128, 128)
        oh = np.ascontiguousarray(
            oh_rows.transpose(1, 0, 2).reshape(128, totch * 128))
        ohT = np.ascontiguousarray(
            oh_rows.transpose(2, 0, 1).reshape(128, totch * 128))
        per_core.append((np.ascontiguousarray(np.tile(ia, (8, 1))),
                         oh, ohT))
    sched = dict(KL=KL, KH=KH, chbase=chbase, colL=colL, colH=colH,
                 totch=totch, totcol=totcol)
    return sched, per_core


def _block_diag_a(a):            # a: [4, 64] -> [256, 4]
    A = np.zeros((HEADS * HID, HEADS), np.float32)
    A[np.arange(HEADS * HID), np.arange(HEADS * HID) // HID] = a.reshape(-1)
    return A


# ------------------------------------------------------------ device program

def _build(s1, s2, win_of):
    nc = bacc.Bacc("TRN2", num_devices=NCORES, num_swdge_queues=4)
    Exp = mybir.ActivationFunctionType.Exp

    d_xs, d_xd, d_dl1 = [], [], []
    for r in range(3):
        t1 = s1[r]["totch"]
        d_xs.append(nc.dram_tensor(f"xs{r}", [128, t1 * 128], BF16,
                                   kind="ExternalInput"))
        d_xd.append(nc.dram_tensor(f"xd{r}", [128, t1 * 128], BF16,
                                   kind="ExternalInput"))
        d_dl1.append(nc.dram_tensor(f"dl1_{r}", [128, t1], F32,
                                    kind="ExternalInput"))
    d_idx, d_oh2, d_ohT = [], [], []
    for r in range(3):
        s = s2[r]
        d_idx.append(nc.dram_tensor(f"idx{r}", [128, s["totcol"]], I16,
                                    kind="ExternalInput"))
        d_oh2.append(nc.dram_tensor(f"oh2_{r}", [128, s["totch"] * 128], BF16,
                                    kind="ExternalInput"))
        d_ohT.append(nc.dram_tensor(f"ohT{r}", [128, s["totch"] * 128], BF16,
                                    kind="ExternalInput"))
    d_w1e = nc.dram_tensor("w1e", [3, IN_CH, MMN], BF16, kind="ExternalInput")
    d_w1ad = nc.dram_tensor("w1ad", [3, IN_CH, HEADS], BF16, kind="ExternalInput")
    d_w2e = nc.dram_tensor("w2e", [3, 2, 128, MMN], BF16, kind="ExternalInput")
    d_w2ad = nc.dram_tensor("w2ad", [3, 2, 128, HEADS], BF16, kind="ExternalInput")
    d_b1b = nc.dram_tensor("b1b", [128, HC], BF16, kind="ExternalInput")
    d_b2b = nc.dram_tensor("b2b", [128, HID], BF16, kind="ExternalInput")
    d_blb = nc.dram_tensor("blb", [128, HID], F32, kind="ExternalInput")
    d_wl = nc.dram_tensor("wl", [HID, HID], BF16, kind="ExternalInput")
    d_iotar = nc.dram_tensor("iotar", [128, 128], BF16, kind="ExternalInput")
    d_iotac = nc.dram_tensor("iotac", [128, 1], BF16, kind="ExternalInput")
    d_identb = nc.dram_tensor("identb", [128, 128], BF16, kind="ExternalInput")

    tab2 = [nc.dram_tensor(f"tab2_{r}", [NPAD, TABW], BF16, kind="Internal")
            for r in range(3)]
    d_x2oT = nc.dram_tensor("x2oT", [2 * 128, NOWN], BF16, kind="Internal")
    d_x2Tf = nc.dram_tensor("x2Tf", [NCORES * 2 * 128, NOWN], BF16,
                            kind="Internal", addr_space="Shared")
    d_out = nc.dram_tensor("out", [NOWN, HID], F32, kind="ExternalOutput")

    qn = [0]    # rotating SWDGE queue

    with tile.TileContext(nc) as tc:
        with (
            tc.tile_pool(name="const", bufs=1) as cp,
            tc.tile_pool(name="acc", bufs=1) as ac,
            tc.tile_pool(name="eio", bufs=2) as ei,
            tc.tile_pool(name="gw", bufs=3) as gp,
            tc.tile_pool(name="st", bufs=2) as sp,
            tc.tile_pool(name="work", bufs=2) as wp,
            tc.tile_pool(name="ps", bufs=2, space="PSUM") as pp,
        ):
            # ---- constants
            t_iotar = cp.tile([128, 128], BF16)
            nc.sync.dma_start(out=t_iotar[:], in_=d_iotar[:])
            t_iotac = cp.tile([128, 1], BF16)
            nc.sync.dma_start(out=t_iotac[:], in_=d_iotac[:])
            t_identb = cp.tile([128, 128], BF16)
            nc.sync.dma_start(out=t_identb[:], in_=d_identb[:])
            t_w1e = [cp.tile([IN_CH, MMN], BF16, tag=f"w1e{r}", name=f"w1e{r}")
                     for r in range(3)]
            t_w1ad = [cp.tile([IN_CH, HEADS], BF16, tag=f"w1ad{r}", name=f"w1ad{r}")
                      for r in range(3)]
            for r in range(3):
                nc.sync.dma_start(out=t_w1e[r][:], in_=d_w1e[r])
                nc.sync.dma_start(out=t_w1ad[r][:], in_=d_w1ad[r])
            t_w2e = [[cp.tile([128, MMN], BF16, tag=f"w2e{r}{k}", name=f"w2e{r}{k}")
                      for k in range(2)] for r in range(3)]
            t_w2ad = [[cp.tile([128, HEADS], BF16, tag=f"w2ad{r}{k}",
                               name=f"w2ad{r}{k}") for k in range(2)]
                      for r in range(3)]
            for r in range(3):
                for k in range(2):
                    nc.sync.dma_start(out=t_w2e[r][k][:], in_=d_w2e[r, k])
                    nc.sync.dma_start(out=t_w2ad[r][k][:], in_=d_w2ad[r, k])
            t_b1b = cp.tile([128, HC], BF16)
            nc.sync.dma_start(out=t_b1b[:], in_=d_b1b[:])
            t_b2b = cp.tile([128, HID], BF16)
            nc.sync.dma_start(out=t_b2b[:], in_=d_b2b[:])
            t_blb = cp.tile([128, HID], F32)
            nc.sync.dma_start(out=t_blb[:], in_=d_blb[:])
            t_wl = cp.tile([HID, HID], BF16)
            nc.sync.dma_start(out=t_wl[:], in_=d_wl[:])
            x2acc = ac.tile([128, W * HC], BF16)
            x3acc = ac.tile([128, W * HID], BF16)

            K1max = max(int(s1[r]["K1"].max()) for r in range(3))
            K2max = max(int((s2[r]["KL"] + s2[r]["KH"]).max()) for r in range(3))

            # ---------------- layer-1 edge phase: stream + project ----------
            def edge_phase1(r):
                K1, chb = s1[r]["K1"], s1[r]["chbase"]
                t_dl = ei.tile([128, s1[r]["totch"]], F32, tag="dl1")
                nc.sync.dma_start(out=t_dl[:], in_=d_dl1[r][:])
                for w in range(W):
                    K = int(K1[w])
                    c0 = int(chb[w])
                    xs = sp.tile([128, K1max * 128], BF16, tag="xs", name="xs")
                    nc.sync.dma_start(
                        out=xs[:, 0:K * 128],
                        in_=d_xs[r][:, c0 * 128:(c0 + K) * 128])
                    xd = sp.tile([128, K1max * 128], BF16, tag="xd", name="xd")
                    nc.sync.dma_start(
                        out=xd[:, 0:K * 128],
                        in_=d_xd[r][:, c0 * 128:(c0 + K) * 128])
                    po = pp.tile([128, MMN], F32, space="PSUM", tag="po")
                    for c in range(K):
                        ph = pp.tile([128, MMN], F32, space="PSUM", tag="ph")
                        nc.tensor.matmul(out=ph[:], lhsT=xs[:, c * 128:(c + 1) * 128],
                                         rhs=t_w1e[r][:], start=True, stop=False,
                                         skip_group_check=True)
                        nc.tensor.matmul(out=ph[:, HC:MMN],
                                         lhsT=xd[:, c * 128:(c + 1) * 128],
                                         rhs=t_w1ad[r][:], start=False, stop=True,
                                         skip_group_check=True)
                        v = wp.tile([128, MMN], BF16, tag="v", bufs=4)
                        e1 = wp.tile([128, HEADS], F32, tag="e1", bufs=4)
                        e2 = wp.tile([128, HEADS], F32, tag="e2", bufs=4)
                        # ex = exp(leaky(z)) = max(exp(z), exp(0.2 z))
                        nc.scalar.activation(out=e1[:], in_=ph[:, HC:MMN],
                                             func=Exp)
                        nc.scalar.activation(out=e2[:], in_=ph[:, HC:MMN],
                                             func=Exp, scale=0.2)
                        nc.vector.tensor_tensor(out=e1[:], in0=e1[:],
                                                in1=e2[:], op=mybir.AluOpType.max)
                        nc.scalar.copy(out=v[:, HC:MMN], in_=e1[:])
                        hsb = wp.tile([128, HC], BF16, tag="hsb", bufs=4)
                        nc.scalar.copy(out=hsb[:], in_=ph[:, 0:HC])
                        for h in range(HEADS):
                            eng = nc.vector if h < 2 else nc.gpsimd
                            eng.tensor_scalar(
                                out=v[:, h * HID:(h + 1) * HID],
                                in0=hsb[:, h * HID:(h + 1) * HID],
                                scalar1=e1[:, h:h + 1], scalar2=None,
                                op0=mybir.AluOpType.mult)
                        oh = wp.tile([128, 128], BF16, tag="oh", bufs=4)
                        nc.gpsimd.tensor_scalar(
                            out=oh[:], in0=t_iotar[:],
                            scalar1=t_dl[:, c0 + c:c0 + c + 1], scalar2=None,
                            op0=mybir.AluOpType.is_equal)
                        nc.tensor.matmul(out=po[:], lhsT=oh[:], rhs=v[:],
                                         start=(c == 0), stop=(c == K - 1))
                    _epilogue1(r, w, po)

            def _epilogue1(r, w, po):
                pon = wp.tile([128, HC], BF16, tag="pon", bufs=3)
                nc.vector.tensor_copy(out=pon[:], in_=po[:, 0:HC])
                rd = wp.tile([128, HEADS], F32, tag="rd", bufs=3)
                nc.vector.tensor_scalar(
                    out=rd[:], in0=po[:, HC:MMN], scalar1=1e-16,
                    scalar2=None, op0=mybir.AluOpType.add)
                nc.vector.reciprocal(out=rd[:], in_=rd[:])
                dstap = x2acc[:, w * HC:(w + 1) * HC]
                if r == 0:
                    for h in range(HEADS):
                        nc.vector.tensor_scalar(
                            out=dstap[:, h * HID:(h + 1) * HID],
                            in0=pon[:, h * HID:(h + 1) * HID],
                            scalar1=rd[:, h:h + 1], scalar2=None,
                            op0=mybir.AluOpType.mult)
                else:
                    tmp = wp.tile([128, HC], BF16, tag="tmp")
                    for h in range(HEADS):
                        nc.scalar.mul(
                            out=tmp[:, h * HID:(h + 1) * HID],
                            in_=pon[:, h * HID:(h + 1) * HID],
                            mul=rd[:, h:h + 1])
                    nc.vector.tensor_tensor(
                        out=dstap, in0=dstap, in1=tmp[:],
                        op=mybir.AluOpType.add)

            # ---------------- layer-2 dense phase: h2 tables ----------------
            def dense_phase2(r):
                DB = 4
                for co in range(NCORES):
                    for so0 in range(0, W, DB):
                        nb = min(DB, W - so0)
                        lhs4 = []
                        for k in range(2):
                            l4 = wp.tile([128, DB * 128], BF16,
                                         tag=f"lhs4{k}", bufs=2, name=f"l4{k}")
                            nc.sync.dma_start(
                                out=l4[:, 0:nb * 128],
                                in_=d_x2Tf[co * 256 + k * 128:
                                           co * 256 + (k + 1) * 128,
                                           so0 * 128:(so0 + nb) * 128])
                            lhs4.append(l4)
                        for i in range(nb):
                            t = int(win_of[co][so0 + i])
                            ph = pp.tile([128, MMN], F32, space="PSUM", tag="ph")
                            for k in range(2):
                                nc.tensor.matmul(
                                    out=ph[:], lhsT=lhs4[k][:, i * 128:(i + 1) * 128],
                                    rhs=t_w2e[r][k][:],
                                    start=(k == 0), stop=(k == 1))
                            hsb = wp.tile([128, MMN], BF16, tag="hsbd", bufs=3)
                            nc.scalar.copy(out=hsb[:], in_=ph[:])
                            nc.sync.dma_start(
                                out=tab2[r][t * 128:(t + 1) * 128, 0:MMN],
                                in_=hsb[:])

            # ---------------- layer-2 edge phase: gather ------------------
            def edge_phase2(r):
                s = s2[r]
                KL, KH = s["KL"], s["KH"]
                chb, colL, colH = s["chbase"], s["colL"], s["colH"]
                t_idx = ei.tile([128, s["totcol"]], I16, tag="idx")
                nc.sync.dma_start(out=t_idx[:], in_=d_idx[r][:])
                for w in range(W):
                    kl, kh = int(KL[w]), int(KH[w])
                    K = kl + kh
                    c0 = int(chb[w])
                    # gathers
                    g = gp.tile([128, K, TABW], BF16, tag="gw")
                    for grp, (kk, coff, base) in enumerate(
                            ((kl, int(colL[w]), 0), (kh, int(colH[w]), kl))):
                        src_ap = tab2[r][:] if grp == 0 else tab2[r][SPLIT:NPAD, :]
                        for sub in range(0, kk, GMAX):
                            nk = min(GMAX, kk - sub)
                            nc.gpsimd.dma_gather(
                                g[:, base + sub:base + sub + nk, :], src_ap,
                                t_idx[:, coff + sub * 8:coff + (sub + nk) * 8],
                                nk * 128, nk * 128, TABW,
                                queue_num=qn[0] % 4)
                            qn[0] += 1
                    # host-built one-hots for the whole window
                    t_oh = gp.tile([128, K2max, 128], BF16, tag="ohL",
                                   bufs=2, name="ohL")
                    nc.sync.dma_start(
                        out=t_oh[:, 0:K, :].rearrange("p k d -> p (k d)"),
                        in_=d_oh2[r][:, c0 * 128:(c0 + K) * 128])
                    ohT = gp.tile([128, K2max, 128], BF16, tag="ohT",
                                  bufs=2, name="ohT")
                    nc.sync.dma_start(
                        out=ohT[:, 0:K, :].rearrange("p k d -> p (k d)"),
                        in_=d_ohT[r][:, c0 * 128:(c0 + K) * 128])
                    # al_d for this window's own 128 dst rows
                    x2w = wp.tile([128, 2, 128], BF16, tag="x2w", bufs=2)
                    for k in range(2):
                        nc.sync.dma_start(
                            out=x2w[:, k, :],
                            in_=d_x2oT[k * 128:(k + 1) * 128,
                                       w * 128:(w + 1) * 128])
                    paw = pp.tile([128, HEADS], F32, space="PSUM", tag="pT")
                    for k in range(2):
                        nc.tensor.matmul(
                            out=paw[:], lhsT=x2w[:, k, :],
                            rhs=t_w2ad[r][k][:],
                            start=(k == 0), stop=(k == 1))
                    aw = wp.tile([128, HEADS], BF16, tag="aw")
                    nc.vector.tensor_copy(out=aw[:], in_=paw[:])
                    po = pp.tile([128, MMN], F32, space="PSUM", tag="po")
                    for c in range(K):
                        pald = pp.tile([128, HEADS], F32, space="PSUM", tag="pald")
                        nc.tensor.matmul(out=pald[:], lhsT=ohT[:, c, :],
                                         rhs=aw[:], start=True, stop=True)
                        v = wp.tile([128, MMN], BF16, tag="v", bufs=4)
                        e1 = wp.tile([128, HEADS], F32, tag="e1", bufs=4)
                        e2 = wp.tile([128, HEADS], F32, tag="e2", bufs=4)
                        zt = wp.tile([128, HEADS], BF16, tag="zt", bufs=4)
                        nc.vector.tensor_tensor(
                            out=zt[:], in0=g[:, c, HC:MMN], in1=pald[:],
                            op=mybir.AluOpType.add)
                        nc.scalar.activation(out=e1[:], in_=zt[:], func=Exp)
                        nc.scalar.activation(out=e2[:], in_=zt[:], func=Exp,
                                             scale=0.2)
                        nc.vector.tensor_tensor(out=e1[:], in0=e1[:],
                                                in1=e2[:], op=mybir.AluOpType.max)
                        nc.scalar.copy(out=v[:, HC:MMN], in_=e1[:])
                        for h in range(HEADS):
                            if h < 2:
                                nc.vector.tensor_scalar(
                                    out=v[:, h * HID:(h + 1) * HID],
                                    in0=g[:, c, h * HID:(h + 1) * HID],
                                    scalar1=e1[:, h:h + 1], scalar2=None,
                                    op0=mybir.AluOpType.mult)
                            else:
                                nc.scalar.mul(
                                    out=v[:, h * HID:(h + 1) * HID],
                                    in_=g[:, c, h * HID:(h + 1) * HID],
                                    mul=e1[:, h:h + 1])
                        nc.tensor.matmul(out=po[:], lhsT=t_oh[:, c, :], rhs=v[:],
                                         start=(c == 0), stop=(c == K - 1))
                    _epilogue2(r, w, po)

            def _epilogue2(r, w, po):
                pon = wp.tile([128, HC], BF16, tag="pon", bufs=3)
                nc.vector.tensor_copy(out=pon[:], in_=po[:, 0:HC])
                rd = wp.tile([128, HEADS], F32, tag="rd", bufs=3)
                nc.vector.tensor_scalar(
                    out=rd[:], in0=po[:, HC:MMN], scalar1=1e-16,
                    scalar2=None, op0=mybir.AluOpType.add)
                nc.vector.reciprocal(out=rd[:], in_=rd[:])
                nc.vector.tensor_scalar(
                    out=rd[:], in0=rd[:], scalar1=0.25, scalar2=None,
                    op0=mybir.AluOpType.mult)
                dstap = x3acc[:, w * HID:(w + 1) * HID]
                for h in range(HEADS):
                    if r == 0 and h == 0:
                        nc.vector.tensor_scalar(
                            out=dstap, in0=pon[:, 0:HID],
                            scalar1=rd[:, 0:1], scalar2=None,
                            op0=mybir.AluOpType.mult)
                    else:
                        t64 = wp.tile([128, HID], BF16, tag="t64")
                        nc.scalar.mul(out=t64[:], in_=pon[:, h * HID:(h + 1) * HID],
                                      mul=rd[:, h:h + 1])
                        nc.vector.tensor_tensor(
                            out=dstap, in0=dstap, in1=t64[:],
                            op=mybir.AluOpType.add)

            # ================= layer 1 =================
            for r in range(3):
                edge_phase1(r)

            # ---- x2 = relu(acc + b1), transpose, store, AllGather
            for w in range(W):
                y = wp.tile([128, HC], BF16, tag="y")
                nc.vector.tensor_tensor(out=y[:], in0=x2acc[:, w * HC:(w + 1) * HC],
                                        in1=t_b1b[:], op=mybir.AluOpType.add)
                nc.vector.tensor_scalar(out=y[:], in0=y[:], scalar1=0.0,
                                        scalar2=None, op0=mybir.AluOpType.max)
                y2T = wp.tile([128, 2, 128], BF16, tag="x2w", bufs=2)
                for k in range(2):
                    psT = pp.tile([128, 128], BF16, space="PSUM", tag="pT")
                    nc.tensor.transpose(out=psT[:], in_=y[:, k * 128:(k + 1) * 128],
                                        identity=t_identb[:])
                    nc.scalar.copy(out=y2T[:, k, :], in_=psT[:])
                    nc.sync.dma_start(
                        out=d_x2oT[k * 128:(k + 1) * 128, w * 128:(w + 1) * 128],
                        in_=y2T[:, k, :])
            nc.gpsimd.collective_compute(
                "AllGather", mybir.AluOpType.bypass,
                replica_groups=[list(range(NCORES))],
                ins=[d_x2oT[:]], outs=[d_x2Tf[:]])

            # ================= layer 2 =================
            for r in range(3):
                dense_phase2(r)
                edge_phase2(r)

            # ---- final: out = relu(x3 + b2) @ Wl + bl
            for w in range(W):
                y = wp.tile([128, HID], BF16, tag="t64")
                nc.vector.tensor_tensor(out=y[:], in0=x3acc[:, w * HID:(w + 1) * HID],
                                        in1=t_b2b[:], op=mybir.AluOpType.add)
                nc.vector.tensor_scalar(out=y[:], in0=y[:], scalar1=0.0,
                                        scalar2=None, op0=mybir.AluOpType.max)
                psT = pp.tile([128, 128], BF16, space="PSUM", tag="pT")
                nc.tensor.transpose(out=psT[:HID, :], in_=y[:], identity=t_identb[:])
                x3T = wp.tile([HID, 128], BF16, tag="x3T")
                nc.scalar.copy(out=x3T[:], in_=psT[:HID, :])
                pf = pp.tile([128, HID], F32, space="PSUM", tag="po")
                nc.tensor.matmul(out=pf[:], lhsT=x3T[:], rhs=t_wl[:],
                                 start=True, stop=True)
                fo = wp.tile([128, HID], F32, tag="fo")
                nc.vector.tensor_tensor(out=fo[:], in0=pf[:], in1=t_blb[:],
                                        op=mybir.AluOpType.add)
                nc.sync.dma_start(out=d_out[w * 128:(w + 1) * 128, :], in_=fo[:])

    nc.compile()
    return nc


# ------------------------------------------------------------------- driver

def kernel(x, e_blocks, e_spatial, e_similar, W1, a1s, a1d, b1,
           W2, a2s, a2d, b2, Wl, bl, **_ignored):
    global last_results
    x = np.asarray(x, np.float32)
    W1 = np.asarray(W1, np.float32)
    a1s = np.asarray(a1s, np.float32)
    a1d = np.asarray(a1d, np.float32)
    b1 = np.asarray(b1, np.float32)
    W2 = np.asarray(W2, np.float32)
    a2s = np.asarray(a2s, np.float32)
    a2d = np.asarray(a2d, np.float32)
    b2 = np.asarray(b2, np.float32)
    Wl = np.asarray(Wl, np.float32)
    bl = np.asarray(bl, np.float32)

    loops = np.arange(N, dtype=np.int64)
    edge_sets = []
    for ei, add_loops in ((e_blocks, False), (e_spatial, True), (e_similar, True)):
        src = np.asarray(ei[0], np.int64)
        dst = np.asarray(ei[1], np.int64)
        if add_loops:
            src = np.concatenate([src, loops])
            dst = np.concatenate([dst, loops])
        edge_sets.append((src, dst))

    win_of, core_of, slot_of = _assign_windows(edge_sets)

    xTb = _bf16(np.concatenate([x.T, np.zeros((IN_CH, NPAD - N), np.float32)],
                               axis=1))
    s1, cores1 = [], []
    s2, cores2 = [], []
    for src, dst in edge_sets:
        sch1, pc1 = _prep_edges_l1(src, dst, core_of, slot_of, xTb)
        s1.append(sch1)
        cores1.append(pc1)
        sch2, pc2 = _prep_edges_l2(src, dst, core_of, slot_of)
        s2.append(sch2)
        cores2.append(pc2)

    w1e = np.zeros((3, IN_CH, MMN), np.float32)
    w1ad = np.zeros((3, IN_CH, HEADS), np.float32)
    w2e = np.zeros((3, 2 * 128, MMN), np.float32)
    w2ad = np.zeros((3, 2 * 128, HEADS), np.float32)
    for r in range(3):
        w1e[r, :, :HC] = W1[r]
        w1e[r, :, HC:MMN] = W1[r] @ _block_diag_a(a1s[r])
        w1ad[r] = W1[r] @ _block_diag_a(a1d[r])
        w2e[r, :, :HC] = W2[r]
        w2e[r, :, HC:MMN] = W2[r] @ _block_diag_a(a2s[r])
        w2ad[r] = W2[r] @ _block_diag_a(a2d[r])
    w2e = w2e.reshape(3, 2, 128, MMN)
    w2ad = w2ad.reshape(3, 2, 128, HEADS)

    common = {
        "w1e": _bf16(w1e), "w1ad": _bf16(w1ad),
        "w2e": _bf16(w2e), "w2ad": _bf16(w2ad),
        "b1b": _bf16(np.tile(b1.sum(0)[None, :], (128, 1))),
        "b2b": _bf16(np.tile(b2.sum(0)[None, :], (128, 1))),
        "blb": np.tile(bl[None, :], (128, 1)).astype(np.float32),
        "wl": _bf16(Wl),
        "iotar": _bf16(np.tile(np.arange(128, dtype=np.float32)[None, :],
                               (128, 1))),
        "iotac": _bf16(np.arange(128, dtype=np.float32)[:, None]),
        "identb": _bf16(np.eye(128, dtype=np.float32)),
    }
    in_maps = []
    for c in range(NCORES):
        m = dict(common)
        for r in range(3):
            xs, xd, dl = cores1[r][c]
            m[f"xs{r}"] = xs
            m[f"xd{r}"] = xd
            m[f"dl1_{r}"] = dl
            ia, oh2, ohT = cores2[r][c]
            m[f"idx{r}"] = ia
            m[f"oh2_{r}"] = oh2
            m[f"ohT{r}"] = ohT
        in_maps.append(m)

    nc = _build(s1, s2, win_of)
    res = run_bass_kernel_spmd(nc, in_maps, core_ids=list(range(NCORES)))
    last_results = res
    full = np.zeros((NPAD, HID), np.float32)
    for c in range(NCORES):
        oc = res.results[c]["out"]
        for s_ in range(W):
            w = int(win_of[c, s_])
            full[w * 128:(w + 1) * 128] = oc[s_ * 128:(s_ + 1) * 128]
    return full[:N].astype(np.float32)


# revision 23
# speedup vs baseline: 3.9863x; 3.9863x over previous
"""HeteroGAT (3-relation, 2-layer GAT + linear head) on 8 Trainium2 cores.

v3: Layer 1 is gather-free: the host pre-permutes x rows into per-edge
order (src and dst streams, f-major), streamed contiguously via HWDGE;
per-chunk projection matmuls produce h|al_s and accumulate al_d into the
same PSUM columns. Layer 2 keeps per-relation h2 tables in HBM with
dma_gather (768B rows), now 3-deep buffered. One-hots are built with
full-rate tensor_scalar is_equal ops (oh) and partition-broadcast
is_equal (ohT); exp(leaky(z)) = max(exp(z), exp(0.2z)) via two ACT ops.
Scatter is a single 260-wide matmul per 128-edge chunk with the softmax
denominators in columns 256:260. Only cross-core exchange: AllGather of
transposed layer-1 activations (bf16).
"""

import numpy as np

import concourse.bacc as bacc
import concourse.bass as bass
import concourse.mybir as mybir
import concourse.tile as tile
from concourse.bass_utils import run_bass_kernel_spmd

F32 = mybir.dt.float32
BF16 = mybir.dt.bfloat16
I16 = mybir.dt.int16

N = 50000
NPAD = 50176            # 392 * 128
NCORES = 8
NOWN = 6272             # 49 * 128 rows per core
W = 49                  # window slots per core
NT = NPAD // 128        # 392 global node tiles / windows
SPLIT = 32768           # int16 index limit for dma_gather
TABW = 384              # L2 table row stride in bf16 elems (768 B, %256)
HC = 256                # feature columns
MMN = 260               # 256 feats + 4 attention cols
IN_CH = 128
HID = 64
HEADS = 4
GMAX = 8                # chunks per dma_gather call (>=2048 idx crashes)

last_results = None


# ----------------------------------------------------------------- host prep

def _bf16(a):
    import ml_dtypes
    return np.asarray(a, np.float32).astype(ml_dtypes.bfloat16)


def _assign_windows(edge_sets):
    """Snake-assign the 392 global windows to (core, slot) by total edge
    count. Returns win_of [NCORES, W] and core_of/slot_of [NT]."""
    score = np.zeros(NT, np.int64)
    for src, dst in edge_sets:
        score += np.bincount(dst >> 7, minlength=NT)
    order = np.argsort(-score, kind="stable")
    win_of = np.zeros((NCORES, W), np.int64)
    for s in range(W):
        grp = order[s * NCORES:(s + 1) * NCORES]
        if s % 2:
            grp = grp[::-1]
        for c in range(NCORES):
            win_of[c, s] = grp[c]
    core_of = np.zeros(NT, np.int64)
    slot_of = np.zeros(NT, np.int64)
    for c in range(NCORES):
        for s in range(W):
            core_of[win_of[c, s]] = c
            slot_of[win_of[c, s]] = s
    return win_of, core_of, slot_of


def _prep_edges_l1(src, dst, core_of, slot_of, xTb):
    """Layer-1 host pre-gather. Edges bucketed by (core, slot); chunk counts
    maxed over cores so the device schedule is common. Returns
    (K1 [W], per_core list of (xs [128, totch*128] bf16,
    xd [...], dl [128, totch] bf16))."""
    E = src.size
    wg = dst >> 7
    sv = slot_of[wg]
    cv = core_of[wg]
    okey = cv * W + sv
    order = np.argsort(okey, kind="stable")
    so, ss, sd = okey[order], src[order], dst[order]
    cnts = np.bincount(okey, minlength=NCORES * W)
    seg_start = np.concatenate([[0], np.cumsum(cnts)[:-1]])
    pos = np.arange(E) - seg_start[so]

    mx = cnts.reshape(NCORES, W).max(axis=0)
    K1 = np.maximum((mx + 127) // 128, 1)
    chbase = np.concatenate([[0], np.cumsum(K1)[:-1]])
    totch = int(K1.sum())

    svs, cvs = sv[order], cv[order]
    col = (chbase[svs] + (pos >> 7)) * 128 + (pos & 127)

    eye_ext = np.zeros((129, 128), np.float32)
    eye_ext[1:] = np.eye(128, dtype=np.float32)
    eye_ext = _bf16(eye_ext)
    per_core = []
    for c in range(NCORES):
        m = cvs == c
        srccols = np.zeros(totch * 128, np.int64)
        dstcols = np.zeros(totch * 128, np.int64)
        dlv = np.full(totch * 128, -1, np.int64)
        srccols[col[m]] = ss[m]
        dstcols[col[m]] = sd[m]
        dlv[col[m]] = sd[m] & 127
        xs = np.ascontiguousarray(xTb[:, srccols])
        xd = np.ascontiguousarray(xTb[:, dstcols])
        oh_rows = eye_ext[dlv + 1].reshape(totch, 128, 128)
        oh = np.ascontiguousarray(
            oh_rows.transpose(1, 0, 2).reshape(128, totch * 128))
        per_core.append((xs, xd, oh))
    return dict(K1=K1, chbase=chbase, totch=totch), per_core


def _prep_edges_l2(src, dst, core_of, slot_of):
    """Layer-2 gather schedule (same as baseline) + dlT row for ohT builds.
    Returns (sched, per_core list of (idx [128, totcol] i16,
    dl [128, totch] bf16, dlT [1, totch*128] bf16))."""
    E = src.size
    wg = dst >> 7
    sv = slot_of[wg]
    cv = core_of[wg]
    gv = (src >= SPLIT).astype(np.int64)
    okey = cv * (2 * W) + sv * 2 + gv
    order = np.argsort(okey, kind="stable")
    so, ss, sd = okey[order], src[order], dst[order]
    cnts = np.bincount(okey, minlength=NCORES * 2 * W)
    seg_start = np.concatenate([[0], np.cumsum(cnts)[:-1]])
    pos = np.arange(E) - seg_start[so]

    mx = cnts.reshape(NCORES, 2 * W).max(axis=0)
    Kg = (mx + 127) // 128
    KL, KH = Kg[0::2].copy(), Kg[1::2].copy()
    KL[(KL + KH) == 0] = 1
    Ksum = KL + KH
    chbase = np.concatenate([[0], np.cumsum(Ksum)[:-1]])
    totch = int(Ksum.sum())
    colL = np.zeros(W, np.int64)
    colH = np.zeros(W, np.int64)
    cum = 0
    for w in range(W):
        colL[w] = cum
        cum += KL[w] * 8
        colH[w] = cum
        cum += KH[w] * 8
    totcol = int(cum)

    svs, gvs, cvs = sv[order], gv[order], cv[order]
    j = np.where(gvs == 0, pos, KL[svs] * 128 + pos)
    ch = chbase[svs] + (j >> 7)
    prow = j & 127
    colbase = np.where(gvs == 0, colL[svs], colH[svs])
    icol = colbase + (pos >> 4)
    irow = pos & 15
    idxval = np.where(gvs == 0, ss, ss - SPLIT).astype(np.int16)
    dloc = (sd & 127).astype(np.float32)

    eye_ext = np.zeros((129, 128), np.float32)
    eye_ext[1:] = np.eye(128, dtype=np.float32)
    eye_ext = _bf16(eye_ext)
    per_core = []
    for c in range(NCORES):
        m = cvs == c
        ia = np.zeros((16, totcol), np.int16)
        ia[irow[m], icol[m]] = idxval[m]
        dlv = np.full(totch * 128, -1, np.int64)
        dlv[ch[m] * 128 + prow[m]] = sd[m] & 127
        oh_rows = eye_ext[dlv + 1].reshape(totch, 128, 128)
        oh = np.ascontiguousarray(
            oh_rows.transpose(1, 0, 2).reshape(128, totch * 128))
        ohT = np.ascontiguousarray(
            oh_rows.transpose(2, 0, 1).reshape(128, totch * 128))
        per_core.append((np.ascontiguousarray(np.tile(ia, (8, 1))),
                         oh, ohT))
    sched = dict(KL=KL, KH=KH, chbase=chbase, colL=colL, colH=colH,
                 totch=totch, totcol=totcol)
    return sched, per_core


def _block_diag_a(a):            # a: [4, 64] -> [256, 4]
    A = np.zeros((HEADS * HID, HEADS), np.float32)
    A[np.arange(HEADS * HID), np.arange(HEADS * HID) // HID] = a.reshape(-1)
    return A


# ------------------------------------------------------------ device program

def _build(s1, s2, win_of):
    nc = bacc.Bacc("TRN2", num_devices=NCORES, num_swdge_queues=4)
    Exp = mybir.ActivationFunctionType.Exp

    d_xs, d_xd, d_oh1 = [], [], []
    for r in range(3):
        t1 = s1[r]["totch"]
        d_xs.append(nc.dram_tensor(f"xs{r}", [128, t1 * 128], BF16,
                                   kind="ExternalInput"))
        d_xd.append(nc.dram_tensor(f"xd{r}", [128, t1 * 128], BF16,
                                   kind="ExternalInput"))
        d_oh1.append(nc.dram_tensor(f"oh1_{r}", [128, t1 * 128], BF16,
                                    kind="ExternalInput"))
    d_idx, d_oh2, d_ohT = [], [], []
    for r in range(3):
        s = s2[r]
        d_idx.append(nc.dram_tensor(f"idx{r}", [128, s["totcol"]], I16,
                                    kind="ExternalInput"))
        d_oh2.append(nc.dram_tensor(f"oh2_{r}", [128, s["totch"] * 128], BF16,
                                    kind="ExternalInput"))
        d_ohT.append(nc.dram_tensor(f"ohT{r}", [128, s["totch"] * 128], BF16,
                                    kind="ExternalInput"))
    d_w1e = nc.dram_tensor("w1e", [3, IN_CH, MMN], BF16, kind="ExternalInput")
    d_w1ad = nc.dram_tensor("w1ad", [3, IN_CH, HEADS], BF16, kind="ExternalInput")
    d_w2e = nc.dram_tensor("w2e", [3, 2, 128, MMN], BF16, kind="ExternalInput")
    d_w2ad = nc.dram_tensor("w2ad", [3, 2, 128, HEADS], BF16, kind="ExternalInput")
    d_b1b = nc.dram_tensor("b1b", [128, HC], BF16, kind="ExternalInput")
    d_b2b = nc.dram_tensor("b2b", [128, HID], BF16, kind="ExternalInput")
    d_blb = nc.dram_tensor("blb", [128, HID], F32, kind="ExternalInput")
    d_wl = nc.dram_tensor("wl", [HID, HID], BF16, kind="ExternalInput")
    d_identb = nc.dram_tensor("identb", [128, 128], BF16, kind="ExternalInput")

    tab2 = [nc.dram_tensor(f"tab2_{r}", [NPAD, TABW], BF16, kind="Internal")
            for r in range(3)]
    d_x2oT = nc.dram_tensor("x2oT", [2 * 128, NOWN], BF16, kind="Internal")
    d_x2Tf = nc.dram_tensor("x2Tf", [NCORES * 2 * 128, NOWN], BF16,
                            kind="Internal", addr_space="Shared")
    d_out = nc.dram_tensor("out", [NOWN, HID], F32, kind="ExternalOutput")

    qn = [0]    # rotating SWDGE queue

    with tile.TileContext(nc) as tc:
        with (
            tc.tile_pool(name="const", bufs=1) as cp,
            tc.tile_pool(name="acc", bufs=1) as ac,
            tc.tile_pool(name="eio", bufs=2) as ei,
            tc.tile_pool(name="gw", bufs=2) as gp,
            tc.tile_pool(name="st", bufs=2) as sp,
            tc.tile_pool(name="work", bufs=2) as wp,
            tc.tile_pool(name="ps", bufs=2, space="PSUM") as pp,
        ):
            # ---- constants
            t_identb = cp.tile([128, 128], BF16)
            nc.sync.dma_start(out=t_identb[:], in_=d_identb[:])
            t_w1e = [cp.tile([IN_CH, MMN], BF16, tag=f"w1e{r}", name=f"w1e{r}")
                     for r in range(3)]
            t_w1ad = [cp.tile([IN_CH, HEADS], BF16, tag=f"w1ad{r}", name=f"w1ad{r}")
                      for r in range(3)]
            for r in range(3):
                nc.sync.dma_start(out=t_w1e[r][:], in_=d_w1e[r])
                nc.sync.dma_start(out=t_w1ad[r][:], in_=d_w1ad[r])
            t_w2e = [[cp.tile([128, MMN], BF16, tag=f"w2e{r}{k}", name=f"w2e{r}{k}")
                      for k in range(2)] for r in range(3)]
            t_w2ad = [[cp.tile([128, HEADS], BF16, tag=f"w2ad{r}{k}",
                               name=f"w2ad{r}{k}") for k in range(2)]
                      for r in range(3)]
            for r in range(3):
                for k in range(2):
                    nc.sync.dma_start(out=t_w2e[r][k][:], in_=d_w2e[r, k])
                    nc.sync.dma_start(out=t_w2ad[r][k][:], in_=d_w2ad[r, k])
            t_b1b = cp.tile([128, HC], BF16)
            nc.sync.dma_start(out=t_b1b[:], in_=d_b1b[:])
            t_b2b = cp.tile([128, HID], BF16)
            nc.sync.dma_start(out=t_b2b[:], in_=d_b2b[:])
            t_blb = cp.tile([128, HID], F32)
            nc.sync.dma_start(out=t_blb[:], in_=d_blb[:])
            t_wl = cp.tile([HID, HID], BF16)
            nc.sync.dma_start(out=t_wl[:], in_=d_wl[:])
            x2acc = ac.tile([128, W * HC], BF16)
            x3acc = ac.tile([128, W * HID], BF16)

            K1max = max(int(s1[r]["K1"].max()) for r in range(3))
            K2max = max(int((s2[r]["KL"] + s2[r]["KH"]).max()) for r in range(3))

            # ---------------- layer-1 edge phase: stream + project ----------
            def edge_phase1(r):
                K1, chb = s1[r]["K1"], s1[r]["chbase"]

                def stage_a(w):
                    K = int(K1[w])
                    c0 = int(chb[w])
                    xs = sp.tile([128, K1max * 128], BF16, tag="xs", name="xs")
                    nc.sync.dma_start(
                        out=xs[:, 0:K * 128],
                        in_=d_xs[r][:, c0 * 128:(c0 + K) * 128])
                    xd = sp.tile([128, K1max * 128], BF16, tag="xd", name="xd")
                    nc.sync.dma_start(
                        out=xd[:, 0:K * 128],
                        in_=d_xd[r][:, c0 * 128:(c0 + K) * 128])
                    oh1 = sp.tile([128, K1max, 128], BF16, tag="oh1", name="oh1")
                    nc.sync.dma_start(
                        out=oh1[:, 0:K, :].rearrange("p k d -> p (k d)"),
                        in_=d_oh1[r][:, c0 * 128:(c0 + K) * 128])
                    hall = wp.tile([128, K1max, MMN], BF16, tag="hall", bufs=2)
                    for c in range(K):
                        ph = pp.tile([128, MMN], F32, space="PSUM", tag="ph")
                        nc.tensor.matmul(out=ph[:], lhsT=xs[:, c * 128:(c + 1) * 128],
                                         rhs=t_w1e[r][:], start=True, stop=False,
                                         skip_group_check=True)
                        nc.tensor.matmul(out=ph[:, HC:MMN],
                                         lhsT=xd[:, c * 128:(c + 1) * 128],
                                         rhs=t_w1ad[r][:], start=False, stop=True,
                                         skip_group_check=True)
                        if c % 2:
                            nc.scalar.copy(out=hall[:, c, :], in_=ph[:])
                        else:
                            nc.vector.tensor_copy(out=hall[:, c, :], in_=ph[:])
                    # batched attention: ex = exp(max(z, 0.2z)); v = h * ex
                    zl = wp.tile([128, K1max, HEADS], BF16, tag="zl", bufs=2)
                    nc.vector.scalar_tensor_tensor(
                        out=zl[:, 0:K, :], in0=hall[:, 0:K, HC:MMN], scalar=0.2,
                        in1=hall[:, 0:K, HC:MMN],
                        op0=mybir.AluOpType.mult, op1=mybir.AluOpType.max)
                    e1 = wp.tile([128, K1max, HEADS], F32, tag="e1", bufs=2)
                    nc.scalar.activation(out=e1[:, 0:K, :], in_=zl[:, 0:K, :],
                                         func=Exp)
                    nc.scalar.copy(out=hall[:, 0:K, HC:MMN], in_=e1[:, 0:K, :])
                    for h in range(HEADS):
                        nc.vector.tensor_tensor(
                            out=hall[:, 0:K, h * HID:(h + 1) * HID],
                            in0=hall[:, 0:K, h * HID:(h + 1) * HID],
                            in1=e1[:, 0:K, h:h + 1].to_broadcast([128, K, HID]),
                            op=mybir.AluOpType.mult)
                    return (w, K, oh1, hall)

                def stage_b(st):
                    w, K, oh1, hall = st
                    po = pp.tile([128, MMN], F32, space="PSUM", tag="po")
                    for c in range(K):
                        nc.tensor.matmul(out=po[:], lhsT=oh1[:, c, :],
                                         rhs=hall[:, c, :],
                                         start=(c == 0), stop=(c == K - 1))
                    _epilogue1(r, w, po)

                prev = None
                for w in range(W):
                    st = stage_a(w)
                    if prev is not None:
                        stage_b(prev)
                    prev = st
                stage_b(prev)

            def _epilogue1(r, w, po):
                pon = wp.tile([128, HC], BF16, tag="pon", bufs=3)
                nc.vector.tensor_copy(out=pon[:], in_=po[:, 0:HC])
                rd = wp.tile([128, HEADS], F32, tag="rd", bufs=3)
                nc.vector.tensor_scalar(
                    out=rd[:], in0=po[:, HC:MMN], scalar1=1e-16,
                    scalar2=None, op0=mybir.AluOpType.add)
                nc.vector.reciprocal(out=rd[:], in_=rd[:])
                dstap = x2acc[:, w * HC:(w + 1) * HC]
                if r == 0:
                    for h in range(HEADS):
                        nc.vector.tensor_scalar(
                            out=dstap[:, h * HID:(h + 1) * HID],
                            in0=pon[:, h * HID:(h + 1) * HID],
                            scalar1=rd[:, h:h + 1], scalar2=None,
                            op0=mybir.AluOpType.mult)
                else:
                    tmp = wp.tile([128, HC], BF16, tag="tmp")
                    for h in range(HEADS):
                        nc.scalar.mul(
                            out=tmp[:, h * HID:(h + 1) * HID],
                            in_=pon[:, h * HID:(h + 1) * HID],
                            mul=rd[:, h:h + 1])
                    nc.vector.tensor_tensor(
                        out=dstap, in0=dstap, in1=tmp[:],
                        op=mybir.AluOpType.add)

            # ---------------- layer-2 dense phase: h2 tables ----------------
            def dense_phase2(r):
                DB = 4
                for co in range(NCORES):
                    for so0 in range(0, W, DB):
                        nb = min(DB, W - so0)
                        lhs4 = []
                        for k in range(2):
                            l4 = wp.tile([128, DB * 128], BF16,
                                         tag=f"lhs4{k}", bufs=2, name=f"l4{k}")
                            nc.sync.dma_start(
                                out=l4[:, 0:nb * 128],
                                in_=d_x2Tf[co * 256 + k * 128:
                                           co * 256 + (k + 1) * 128,
                                           so0 * 128:(so0 + nb) * 128])
                            lhs4.append(l4)
                        for i in range(nb):
                            t = int(win_of[co][so0 + i])
                            ph = pp.tile([128, MMN], F32, space="PSUM", tag="ph")
                            for k in range(2):
                                nc.tensor.matmul(
                                    out=ph[:], lhsT=lhs4[k][:, i * 128:(i + 1) * 128],
                                    rhs=t_w2e[r][k][:],
                                    start=(k == 0), stop=(k == 1))
                            hsb = wp.tile([128, MMN], BF16, tag="hsbd", bufs=3)
                            nc.scalar.copy(out=hsb[:], in_=ph[:])
                            nc.sync.dma_start(
                                out=tab2[r][t * 128:(t + 1) * 128, 0:MMN],
                                in_=hsb[:])

            # ---------------- layer-2 edge phase: gather ------------------
            def edge_phase2(r):
                s = s2[r]
                KL, KH = s["KL"], s["KH"]
                chb, colL, colH = s["chbase"], s["colL"], s["colH"]
                t_idx = ei.tile([128, s["totcol"]], I16, tag="idx")
                nc.sync.dma_start(out=t_idx[:], in_=d_idx[r][:])

                def stage_a(w):
                    kl, kh = int(KL[w]), int(KH[w])
                    K = kl + kh
                    c0 = int(chb[w])
                    g = gp.tile([128, K2max, TABW], BF16, tag="gw")
                    for grp, (kk, coff, base) in enumerate(
                            ((kl, int(colL[w]), 0), (kh, int(colH[w]), kl))):
                        src_ap = tab2[r][:] if grp == 0 else tab2[r][SPLIT:NPAD, :]
                        for sub in range(0, kk, GMAX):
                            nk = min(GMAX, kk - sub)
                            nc.gpsimd.dma_gather(
                                g[:, base + sub:base + sub + nk, :], src_ap,
                                t_idx[:, coff + sub * 8:coff + (sub + nk) * 8],
                                nk * 128, nk * 128, TABW,
                                queue_num=qn[0] % 4)
                            qn[0] += 1
                    t_oh = gp.tile([128, K2max, 128], BF16, tag="ohL",
                                   bufs=2, name="ohL")
                    nc.sync.dma_start(
                        out=t_oh[:, 0:K, :].rearrange("p k d -> p (k d)"),
                        in_=d_oh2[r][:, c0 * 128:(c0 + K) * 128])
                    ohT = gp.tile([128, K2max, 128], BF16, tag="ohT",
                                  bufs=2, name="ohT")
                    nc.sync.dma_start(
                        out=ohT[:, 0:K, :].rearrange("p k d -> p (k d)"),
                        in_=d_ohT[r][:, c0 * 128:(c0 + K) * 128])
                    # al_d for this window's own 128 dst rows
                    x2w = wp.tile([128, 2, 128], BF16, tag="x2w", bufs=2)
                    for k in range(2):
                        nc.sync.dma_start(
                            out=x2w[:, k, :],
                            in_=d_x2oT[k * 128:(k + 1) * 128,
                                       w * 128:(w + 1) * 128])
                    paw = pp.tile([128, HEADS], F32, space="PSUM", tag="pT")
                    for k in range(2):
                        nc.tensor.matmul(
                            out=paw[:], lhsT=x2w[:, k, :],
                            rhs=t_w2ad[r][k][:],
                            start=(k == 0), stop=(k == 1))
                    aw = wp.tile([128, HEADS], BF16, tag="aw")
                    nc.vector.tensor_copy(out=aw[:], in_=paw[:])
                    # per-edge al_d for all chunks, then batched attention
                    pz = pp.tile([128, K2max, HEADS], F32, space="PSUM", tag="pald")
                    for c in range(K):
                        nc.tensor.matmul(out=pz[:, c, :], lhsT=ohT[:, c, :],
                                         rhs=aw[:], start=True, stop=True,
                                         skip_group_check=True)
                    zl = wp.tile([128, K2max, HEADS], BF16, tag="zl", bufs=2)
                    nc.vector.tensor_tensor(
                        out=zl[:, 0:K, :], in0=g[:, 0:K, HC:MMN],
                        in1=pz[:, 0:K, :], op=mybir.AluOpType.add)
                    nc.vector.scalar_tensor_tensor(
                        out=zl[:, 0:K, :], in0=zl[:, 0:K, :], scalar=0.2,
                        in1=zl[:, 0:K, :],
                        op0=mybir.AluOpType.mult, op1=mybir.AluOpType.max)
                    e1 = wp.tile([128, K2max, HEADS], F32, tag="e1", bufs=2)
                    nc.scalar.activation(out=e1[:, 0:K, :], in_=zl[:, 0:K, :],
                                         func=Exp)
                    nc.scalar.copy(out=g[:, 0:K, HC:MMN], in_=e1[:, 0:K, :])
                    for h in range(HEADS):
                        nc.vector.tensor_tensor(
                            out=g[:, 0:K, h * HID:(h + 1) * HID],
                            in0=g[:, 0:K, h * HID:(h + 1) * HID],
                            in1=e1[:, 0:K, h:h + 1].to_broadcast([128, K, HID]),
                            op=mybir.AluOpType.mult)
                    return (w, K, t_oh, g)

                def stage_b(st):
                    w, K, t_oh, g = st
                    po = pp.tile([128, MMN], F32, space="PSUM", tag="po")
                    for c in range(K):
                        nc.tensor.matmul(out=po[:], lhsT=t_oh[:, c, :],
                                         rhs=g[:, c, 0:MMN],
                                         start=(c == 0), stop=(c == K - 1))
                    _epilogue2(r, w, po)

                prev = None
                for w in range(W):
                    st = stage_a(w)
                    if prev is not None:
                        stage_b(prev)
                    prev = st
                stage_b(prev)

            def _epilogue2(r, w, po):
                pon = wp.tile([128, HC], BF16, tag="pon", bufs=3)
                nc.vector.tensor_copy(out=pon[:], in_=po[:, 0:HC])
                rd = wp.tile([128, HEADS], F32, tag="rd", bufs=3)
                nc.vector.tensor_scalar(
                    out=rd[:], in0=po[:, HC:MMN], scalar1=1e-16,
                    scalar2=None, op0=mybir.AluOpType.add)
                nc.vector.reciprocal(out=rd[:], in_=rd[:])
                nc.vector.tensor_scalar(
                    out=rd[:], in0=rd[:], scalar1=0.25, scalar2=None,
                    op0=mybir.AluOpType.mult)
                dstap = x3acc[:, w * HID:(w + 1) * HID]
                for h in range(HEADS):
                    if r == 0 and h == 0:
                        nc.vector.tensor_scalar(
                            out=dstap, in0=pon[:, 0:HID],
                            scalar1=rd[:, 0:1], scalar2=None,
                            op0=mybir.AluOpType.mult)
                    else:
                        t64 = wp.tile([128, HID], BF16, tag="t64")
                        nc.scalar.mul(out=t64[:], in_=pon[:, h * HID:(h + 1) * HID],
                                      mul=rd[:, h:h + 1])
                        nc.vector.tensor_tensor(
                            out=dstap, in0=dstap, in1=t64[:],
                            op=mybir.AluOpType.add)

            # ================= layer 1 =================
            for r in range(3):
                edge_phase1(r)

            # ---- x2 = relu(acc + b1), transpose, store, AllGather
            for w in range(W):
                y = wp.tile([128, HC], BF16, tag="y")
                nc.vector.tensor_tensor(out=y[:], in0=x2acc[:, w * HC:(w + 1) * HC],
                                        in1=t_b1b[:], op=mybir.AluOpType.add)
                nc.vector.tensor_scalar(out=y[:], in0=y[:], scalar1=0.0,
                                        scalar2=None, op0=mybir.AluOpType.max)
                y2T = wp.tile([128, 2, 128], BF16, tag="x2w", bufs=2)
                for k in range(2):
                    psT = pp.tile([128, 128], BF16, space="PSUM", tag="pT")
                    nc.tensor.transpose(out=psT[:], in_=y[:, k * 128:(k + 1) * 128],
                                        identity=t_identb[:])
                    nc.scalar.copy(out=y2T[:, k, :], in_=psT[:])
                    nc.sync.dma_start(
                        out=d_x2oT[k * 128:(k + 1) * 128, w * 128:(w + 1) * 128],
                        in_=y2T[:, k, :])
            nc.gpsimd.collective_compute(
                "AllGather", mybir.AluOpType.bypass,
                replica_groups=[list(range(NCORES))],
                ins=[d_x2oT[:]], outs=[d_x2Tf[:]])

            # ================= layer 2 =================
            for r in range(3):
                dense_phase2(r)
                edge_phase2(r)

            # ---- final: out = relu(x3 + b2) @ Wl + bl
            for w in range(W):
                y = wp.tile([128, HID], BF16, tag="t64")
                nc.vector.tensor_tensor(out=y[:], in0=x3acc[:, w * HID:(w + 1) * HID],
                                        in1=t_b2b[:], op=mybir.AluOpType.add)
                nc.vector.tensor_scalar(out=y[:], in0=y[:], scalar1=0.0,
                                        scalar2=None, op0=mybir.AluOpType.max)
                psT = pp.tile([128, 128], BF16, space="PSUM", tag="pT")
                nc.tensor.transpose(out=psT[:HID, :], in_=y[:], identity=t_identb[:])
                x3T = wp.tile([HID, 128], BF16, tag="x3T")
                nc.scalar.copy(out=x3T[:], in_=psT[:HID, :])
                pf = pp.tile([128, HID], F32, space="PSUM", tag="po")
                nc.tensor.matmul(out=pf[:], lhsT=x3T[:], rhs=t_wl[:],
                                 start=True, stop=True)
                fo = wp.tile([128, HID], F32, tag="fo")
                nc.vector.tensor_tensor(out=fo[:], in0=pf[:], in1=t_blb[:],
                                        op=mybir.AluOpType.add)
                nc.sync.dma_start(out=d_out[w * 128:(w + 1) * 128, :], in_=fo[:])

    nc.compile()
    return nc


# ------------------------------------------------------------------- driver

def kernel(x, e_blocks, e_spatial, e_similar, W1, a1s, a1d, b1,
           W2, a2s, a2d, b2, Wl, bl, **_ignored):
    global last_results
    x = np.asarray(x, np.float32)
    W1 = np.asarray(W1, np.float32)
    a1s = np.asarray(a1s, np.float32)
    a1d = np.asarray(a1d, np.float32)
    b1 = np.asarray(b1, np.float32)
    W2 = np.asarray(W2, np.float32)
    a2s = np.asarray(a2s, np.float32)
    a2d = np.asarray(a2d, np.float32)
    b2 = np.asarray(b2, np.float32)
    Wl = np.asarray(Wl, np.float32)
    bl = np.asarray(bl, np.float32)

    loops = np.arange(N, dtype=np.int64)
    edge_sets = []
    for ei, add_loops in ((e_blocks, False), (e_spatial, True), (e_similar, True)):
        src = np.asarray(ei[0], np.int64)
        dst = np.asarray(ei[1], np.int64)
        if add_loops:
            src = np.concatenate([src, loops])
            dst = np.concatenate([dst, loops])
        edge_sets.append((src, dst))

    win_of, core_of, slot_of = _assign_windows(edge_sets)

    xTb = _bf16(np.concatenate([x.T, np.zeros((IN_CH, NPAD - N), np.float32)],
                               axis=1))
    s1, cores1 = [], []
    s2, cores2 = [], []
    for src, dst in edge_sets:
        sch1, pc1 = _prep_edges_l1(src, dst, core_of, slot_of, xTb)
        s1.append(sch1)
        cores1.append(pc1)
        sch2, pc2 = _prep_edges_l2(src, dst, core_of, slot_of)
        s2.append(sch2)
        cores2.append(pc2)

    w1e = np.zeros((3, IN_CH, MMN), np.float32)
    w1ad = np.zeros((3, IN_CH, HEADS), np.float32)
    w2e = np.zeros((3, 2 * 128, MMN), np.float32)
    w2ad = np.zeros((3, 2 * 128, HEADS), np.float32)
    for r in range(3):
        w1e[r, :, :HC] = W1[r]
        w1e[r, :, HC:MMN] = W1[r] @ _block_diag_a(a1s[r])
        w1ad[r] = W1[r] @ _block_diag_a(a1d[r])
        w2e[r, :, :HC] = W2[r]
        w2e[r, :, HC:MMN] = W2[r] @ _block_diag_a(a2s[r])
        w2ad[r] = W2[r] @ _block_diag_a(a2d[r])
    w2e = w2e.reshape(3, 2, 128, MMN)
    w2ad = w2ad.reshape(3, 2, 128, HEADS)

    common = {
        "w1e": _bf16(w1e), "w1ad": _bf16(w1ad),
        "w2e": _bf16(w2e), "w2ad": _bf16(w2ad),
        "b1b": _bf16(np.tile(b1.sum(0)[None, :], (128, 1))),
        "b2b": _bf16(np.tile(b2.sum(0)[None, :], (128, 1))),
        "blb": np.tile(bl[None, :], (128, 1)).astype(np.float32),
        "wl": _bf16(Wl),
        "identb": _bf16(np.eye(128, dtype=np.float32)),
    }
    in_maps = []
    for c in range(NCORES):
        m = dict(common)
        for r in range(3):
            xs, xd, oh1 = cores1[r][c]
            m[f"xs{r}"] = xs
            m[f"xd{r}"] = xd
            m[f"oh1_{r}"] = oh1
            ia, oh2, ohT = cores2[r][c]
            m[f"idx{r}"] = ia
            m[f"oh2_{r}"] = oh2
            m[f"ohT{r}"] = ohT
        in_maps.append(m)

    nc = _build(s1, s2, win_of)
    res = run_bass_kernel_spmd(nc, in_maps, core_ids=list(range(NCORES)))
    last_results = res
    full = np.zeros((NPAD, HID), np.float32)
    for c in range(NCORES):
        oc = res.results[c]["out"]
        for s_ in range(W):
            w = int(win_of[c, s_])
            full[w * 128:(w + 1) * 128] = oc[s_ * 128:(s_ + 1) * 128]
    return full[:N].astype(np.float32)


# revision 26
# speedup vs baseline: 4.4355x; 1.1127x over previous
"""HeteroGAT (3-relation, 2-layer GAT + linear head) on 8 Trainium2 cores.

v3: Layer 1 is gather-free: the host pre-permutes x rows into per-edge
order (src and dst streams, f-major), streamed contiguously via HWDGE;
per-chunk projection matmuls produce h|al_s and accumulate al_d into the
same PSUM columns. Layer 2 keeps per-relation h2 tables in HBM with
dma_gather (768B rows), now 3-deep buffered. One-hots are built with
full-rate tensor_scalar is_equal ops (oh) and partition-broadcast
is_equal (ohT); exp(leaky(z)) = max(exp(z), exp(0.2z)) via two ACT ops.
Scatter is a single 260-wide matmul per 128-edge chunk with the softmax
denominators in columns 256:260. Only cross-core exchange: AllGather of
transposed layer-1 activations (bf16).
"""

import numpy as np

import concourse.bacc as bacc
import concourse.bass as bass
import concourse.mybir as mybir
import concourse.tile as tile
from concourse.bass_utils import run_bass_kernel_spmd

F32 = mybir.dt.float32
BF16 = mybir.dt.bfloat16
I16 = mybir.dt.int16

N = 50000
NPAD = 50176            # 392 * 128
NCORES = 8
NOWN = 6272             # 49 * 128 rows per core
W = 49                  # window slots per core
NT = NPAD // 128        # 392 global node tiles / windows
SPLIT = 32768           # int16 index limit for dma_gather
TABW = 384              # L2 table row stride in bf16 elems (768 B, %256)
HC = 256                # feature columns
MMN = 260               # 256 feats + 4 attention cols
IN_CH = 128
HID = 64
HEADS = 4
GMAX = 8                # chunks per dma_gather call (>=2048 idx crashes)

last_results = None


# ----------------------------------------------------------------- host prep

def _bf16(a):
    import ml_dtypes
    return np.asarray(a, np.float32).astype(ml_dtypes.bfloat16)


def _assign_windows(edge_sets):
    """Snake-assign the 392 global windows to (core, slot) by total edge
    count. Returns win_of [NCORES, W] and core_of/slot_of [NT]."""
    score = np.zeros(NT, np.int64)
    for src, dst in edge_sets:
        score += np.bincount(dst >> 7, minlength=NT)
    order = np.argsort(-score, kind="stable")
    win_of = np.zeros((NCORES, W), np.int64)
    for s in range(W):
        grp = order[s * NCORES:(s + 1) * NCORES]
        if s % 2:
            grp = grp[::-1]
        for c in range(NCORES):
            win_of[c, s] = grp[c]
    core_of = np.zeros(NT, np.int64)
    slot_of = np.zeros(NT, np.int64)
    for c in range(NCORES):
        for s in range(W):
            core_of[win_of[c, s]] = c
            slot_of[win_of[c, s]] = s
    return win_of, core_of, slot_of


def _prep_edges_l1(src, dst, core_of, slot_of, xTb):
    """Layer-1 host pre-gather. Edges bucketed by (core, slot); chunk counts
    maxed over cores so the device schedule is common. Returns
    (K1 [W], per_core list of (xs [128, totch*128] bf16,
    xd [...], dl [128, totch] bf16))."""
    E = src.size
    wg = dst >> 7
    sv = slot_of[wg]
    cv = core_of[wg]
    okey = cv * W + sv
    order = np.argsort(okey, kind="stable")
    so, ss, sd = okey[order], src[order], dst[order]
    cnts = np.bincount(okey, minlength=NCORES * W)
    seg_start = np.concatenate([[0], np.cumsum(cnts)[:-1]])
    pos = np.arange(E) - seg_start[so]

    mx = cnts.reshape(NCORES, W).max(axis=0)
    K1 = np.maximum((mx + 127) // 128, 1)
    chbase = np.concatenate([[0], np.cumsum(K1)[:-1]])
    totch = int(K1.sum())

    svs, cvs = sv[order], cv[order]
    col = (chbase[svs] + (pos >> 7)) * 128 + (pos & 127)

    eye_ext = np.zeros((129, 128), np.float32)
    eye_ext[1:] = np.eye(128, dtype=np.float32)
    eye_ext = _bf16(eye_ext)
    per_core = []
    for c in range(NCORES):
        m = cvs == c
        srccols = np.zeros(totch * 128, np.int64)
        dstcols = np.zeros(totch * 128, np.int64)
        dlv = np.full(totch * 128, -1, np.int64)
        srccols[col[m]] = ss[m]
        dstcols[col[m]] = sd[m]
        dlv[col[m]] = sd[m] & 127
        xs = np.ascontiguousarray(xTb[:, srccols])
        xd = np.ascontiguousarray(xTb[:, dstcols])
        oh_rows = eye_ext[dlv + 1].reshape(totch, 128, 128)
        oh = np.ascontiguousarray(
            oh_rows.transpose(1, 0, 2).reshape(128, totch * 128))
        per_core.append((xs, xd, oh))
    return dict(K1=K1, chbase=chbase, totch=totch), per_core


def _prep_edges_l2(src, dst, core_of, slot_of):
    """Layer-2 gather schedule (same as baseline) + dlT row for ohT builds.
    Returns (sched, per_core list of (idx [128, totcol] i16,
    dl [128, totch] bf16, dlT [1, totch*128] bf16))."""
    E = src.size
    wg = dst >> 7
    sv = slot_of[wg]
    cv = core_of[wg]
    # gather table rows live in (core, slot) order
    swg = src >> 7
    srcpos = (core_of[swg] * W + slot_of[swg]) * 128 + (src & 127)
    gv = (srcpos >= SPLIT).astype(np.int64)
    okey = cv * (2 * W) + sv * 2 + gv
    order = np.argsort(okey, kind="stable")
    so, ss, sd = okey[order], src[order], dst[order]
    cnts = np.bincount(okey, minlength=NCORES * 2 * W)
    seg_start = np.concatenate([[0], np.cumsum(cnts)[:-1]])
    pos = np.arange(E) - seg_start[so]

    mx = cnts.reshape(NCORES, 2 * W).max(axis=0)
    Kg = (mx + 127) // 128
    KL, KH = Kg[0::2].copy(), Kg[1::2].copy()
    KL[(KL + KH) == 0] = 1
    Ksum = KL + KH
    chbase = np.concatenate([[0], np.cumsum(Ksum)[:-1]])
    totch = int(Ksum.sum())
    colL = np.zeros(W, np.int64)
    colH = np.zeros(W, np.int64)
    cum = 0
    for w in range(W):
        colL[w] = cum
        cum += KL[w] * 8
        colH[w] = cum
        cum += KH[w] * 8
    totcol = int(cum)

    svs, gvs, cvs = sv[order], gv[order], cv[order]
    j = np.where(gvs == 0, pos, KL[svs] * 128 + pos)
    ch = chbase[svs] + (j >> 7)
    prow = j & 127
    colbase = np.where(gvs == 0, colL[svs], colH[svs])
    icol = colbase + (pos >> 4)
    irow = pos & 15
    sp_s = srcpos[order]
    idxval = np.where(gvs == 0, sp_s, sp_s - SPLIT).astype(np.int16)
    dloc = (sd & 127).astype(np.float32)

    eye_ext = np.zeros((129, 128), np.float32)
    eye_ext[1:] = np.eye(128, dtype=np.float32)
    eye_ext = _bf16(eye_ext)
    per_core = []
    for c in range(NCORES):
        m = cvs == c
        ia = np.zeros((16, totcol), np.int16)
        ia[irow[m], icol[m]] = idxval[m]
        dlv = np.full(totch * 128, -1, np.int64)
        dlv[ch[m] * 128 + prow[m]] = sd[m] & 127
        oh_rows = eye_ext[dlv + 1].reshape(totch, 128, 128)
        oh = np.ascontiguousarray(
            oh_rows.transpose(1, 0, 2).reshape(128, totch * 128))
        ohT = np.ascontiguousarray(
            oh_rows.transpose(2, 0, 1).reshape(128, totch * 128))
        per_core.append((np.ascontiguousarray(np.tile(ia, (8, 1))),
                         oh, ohT))
    sched = dict(KL=KL, KH=KH, chbase=chbase, colL=colL, colH=colH,
                 totch=totch, totcol=totcol)
    return sched, per_core


def _block_diag_a(a):            # a: [4, 64] -> [256, 4]
    A = np.zeros((HEADS * HID, HEADS), np.float32)
    A[np.arange(HEADS * HID), np.arange(HEADS * HID) // HID] = a.reshape(-1)
    return A


# ------------------------------------------------------------ device program

def _build(s1, s2, win_of):
    nc = bacc.Bacc("TRN2", num_devices=NCORES, num_swdge_queues=4)
    Exp = mybir.ActivationFunctionType.Exp

    d_xs, d_xd, d_oh1 = [], [], []
    for r in range(3):
        t1 = s1[r]["totch"]
        d_xs.append(nc.dram_tensor(f"xs{r}", [128, t1 * 128], BF16,
                                   kind="ExternalInput"))
        d_xd.append(nc.dram_tensor(f"xd{r}", [128, t1 * 128], BF16,
                                   kind="ExternalInput"))
        d_oh1.append(nc.dram_tensor(f"oh1_{r}", [128, t1 * 128], BF16,
                                    kind="ExternalInput"))
    d_idx, d_oh2, d_ohT = [], [], []
    for r in range(3):
        s = s2[r]
        d_idx.append(nc.dram_tensor(f"idx{r}", [128, s["totcol"]], I16,
                                    kind="ExternalInput"))
        d_oh2.append(nc.dram_tensor(f"oh2_{r}", [128, s["totch"] * 128], BF16,
                                    kind="ExternalInput"))
        d_ohT.append(nc.dram_tensor(f"ohT{r}", [128, s["totch"] * 128], BF16,
                                    kind="ExternalInput"))
    d_w1e = nc.dram_tensor("w1e", [3, IN_CH, MMN], BF16, kind="ExternalInput")
    d_w1ad = nc.dram_tensor("w1ad", [3, IN_CH, HEADS], BF16, kind="ExternalInput")
    d_w2e = nc.dram_tensor("w2e", [3, 2, 128, MMN], BF16, kind="ExternalInput")
    d_w2ad = nc.dram_tensor("w2ad", [3, 2, 128, HEADS], BF16, kind="ExternalInput")
    d_b1b = nc.dram_tensor("b1b", [128, HC], BF16, kind="ExternalInput")
    d_b2b = nc.dram_tensor("b2b", [128, HID], BF16, kind="ExternalInput")
    d_blb = nc.dram_tensor("blb", [128, HID], F32, kind="ExternalInput")
    d_wl = nc.dram_tensor("wl", [HID, HID], BF16, kind="ExternalInput")
    d_identb = nc.dram_tensor("identb", [128, 128], BF16, kind="ExternalInput")

    tab2 = [nc.dram_tensor(f"tab2_{r}", [NPAD, TABW], BF16, kind="Internal")
            for r in range(3)]
    d_x2oT = nc.dram_tensor("x2oT", [2 * 128, NOWN], BF16, kind="Internal")
    d_x2Tf = nc.dram_tensor("x2Tf", [NCORES * 2 * 128, NOWN], BF16,
                            kind="Internal", addr_space="Shared")
    d_out = nc.dram_tensor("out", [NOWN, HID], F32, kind="ExternalOutput")

    qn = [0]    # rotating SWDGE queue

    with tile.TileContext(nc) as tc:
        with (
            tc.tile_pool(name="const", bufs=1) as cp,
            tc.tile_pool(name="acc", bufs=1) as ac,
            tc.tile_pool(name="eio", bufs=2) as ei,
            tc.tile_pool(name="gw", bufs=2) as gp,
            tc.tile_pool(name="st", bufs=2) as sp,
            tc.tile_pool(name="work", bufs=2) as wp,
            tc.tile_pool(name="ps", bufs=2, space="PSUM") as pp,
        ):
            # ---- constants
            t_identb = cp.tile([128, 128], BF16)
            nc.sync.dma_start(out=t_identb[:], in_=d_identb[:])
            t_w1e = [cp.tile([IN_CH, MMN], BF16, tag=f"w1e{r}", name=f"w1e{r}")
                     for r in range(3)]
            t_w1ad = [cp.tile([IN_CH, HEADS], BF16, tag=f"w1ad{r}", name=f"w1ad{r}")
                      for r in range(3)]
            for r in range(3):
                nc.sync.dma_start(out=t_w1e[r][:], in_=d_w1e[r])
                nc.sync.dma_start(out=t_w1ad[r][:], in_=d_w1ad[r])
            t_w2e = [[cp.tile([128, MMN], BF16, tag=f"w2e{r}{k}", name=f"w2e{r}{k}")
                      for k in range(2)] for r in range(3)]
            t_w2ad = [[cp.tile([128, HEADS], BF16, tag=f"w2ad{r}{k}",
                               name=f"w2ad{r}{k}") for k in range(2)]
                      for r in range(3)]
            for r in range(3):
                for k in range(2):
                    nc.sync.dma_start(out=t_w2e[r][k][:], in_=d_w2e[r, k])
                    nc.sync.dma_start(out=t_w2ad[r][k][:], in_=d_w2ad[r, k])
            t_b1b = cp.tile([128, HC], BF16)
            nc.sync.dma_start(out=t_b1b[:], in_=d_b1b[:])
            t_b2b = cp.tile([128, HID], BF16)
            nc.sync.dma_start(out=t_b2b[:], in_=d_b2b[:])
            t_blb = cp.tile([128, HID], F32)
            nc.sync.dma_start(out=t_blb[:], in_=d_blb[:])
            t_wl = cp.tile([HID, HID], BF16)
            nc.sync.dma_start(out=t_wl[:], in_=d_wl[:])
            x2acc = ac.tile([128, W * HC], BF16)
            x3acc = ac.tile([128, W * HID], BF16)

            K1max = max(int(s1[r]["K1"].max()) for r in range(3))
            K2max = max(int((s2[r]["KL"] + s2[r]["KH"]).max()) for r in range(3))

            # ---------------- layer-1 edge phase: stream + project ----------
            def edge_phase1(r):
                K1, chb = s1[r]["K1"], s1[r]["chbase"]

                def stage_a(w):
                    K = int(K1[w])
                    c0 = int(chb[w])
                    xs = sp.tile([128, K1max * 128], BF16, tag="xs", name="xs")
                    nc.sync.dma_start(
                        out=xs[:, 0:K * 128],
                        in_=d_xs[r][:, c0 * 128:(c0 + K) * 128])
                    xd = sp.tile([128, K1max * 128], BF16, tag="xd", name="xd")
                    nc.sync.dma_start(
                        out=xd[:, 0:K * 128],
                        in_=d_xd[r][:, c0 * 128:(c0 + K) * 128])
                    oh1 = sp.tile([128, K1max, 128], BF16, tag="oh1", name="oh1")
                    nc.sync.dma_start(
                        out=oh1[:, 0:K, :].rearrange("p k d -> p (k d)"),
                        in_=d_oh1[r][:, c0 * 128:(c0 + K) * 128])
                    hall = wp.tile([128, K1max, MMN], BF16, tag="hall", bufs=2)
                    for c in range(K):
                        ph = pp.tile([128, MMN], F32, space="PSUM", tag="ph")
                        nc.tensor.matmul(out=ph[:], lhsT=xs[:, c * 128:(c + 1) * 128],
                                         rhs=t_w1e[r][:], start=True, stop=False,
                                         skip_group_check=True)
                        nc.tensor.matmul(out=ph[:, HC:MMN],
                                         lhsT=xd[:, c * 128:(c + 1) * 128],
                                         rhs=t_w1ad[r][:], start=False, stop=True,
                                         skip_group_check=True)
                        if c % 2:
                            nc.scalar.copy(out=hall[:, c, :], in_=ph[:])
                        else:
                            nc.vector.tensor_copy(out=hall[:, c, :], in_=ph[:])
                    # batched attention: ex = exp(max(z, 0.2z)); v = h * ex
                    zl = wp.tile([128, K1max, HEADS], BF16, tag="zl", bufs=2)
                    nc.vector.scalar_tensor_tensor(
                        out=zl[:, 0:K, :], in0=hall[:, 0:K, HC:MMN], scalar=0.2,
                        in1=hall[:, 0:K, HC:MMN],
                        op0=mybir.AluOpType.mult, op1=mybir.AluOpType.max)
                    e1 = wp.tile([128, K1max, HEADS], F32, tag="e1", bufs=2)
                    nc.scalar.activation(out=e1[:, 0:K, :], in_=zl[:, 0:K, :],
                                         func=Exp)
                    nc.scalar.copy(out=hall[:, 0:K, HC:MMN], in_=e1[:, 0:K, :])
                    for h in range(HEADS):
                        nc.vector.tensor_tensor(
                            out=hall[:, 0:K, h * HID:(h + 1) * HID],
                            in0=hall[:, 0:K, h * HID:(h + 1) * HID],
                            in1=e1[:, 0:K, h:h + 1].to_broadcast([128, K, HID]),
                            op=mybir.AluOpType.mult)
                    return (w, K, oh1, hall)

                def stage_b(st):
                    w, K, oh1, hall = st
                    po = pp.tile([128, MMN], F32, space="PSUM", tag="po")
                    for c in range(K):
                        nc.tensor.matmul(out=po[:], lhsT=oh1[:, c, :],
                                         rhs=hall[:, c, :],
                                         start=(c == 0), stop=(c == K - 1))
                    _epilogue1(r, w, po)

                prev = None
                for w in range(W):
                    st = stage_a(w)
                    if prev is not None:
                        stage_b(prev)
                    prev = st
                stage_b(prev)

            def _epilogue1(r, w, po):
                pon = wp.tile([128, HC], BF16, tag="pon", bufs=3)
                nc.vector.tensor_copy(out=pon[:], in_=po[:, 0:HC])
                rd = wp.tile([128, HEADS], F32, tag="rd", bufs=3)
                nc.vector.tensor_scalar(
                    out=rd[:], in0=po[:, HC:MMN], scalar1=1e-16,
                    scalar2=None, op0=mybir.AluOpType.add)
                nc.vector.reciprocal(out=rd[:], in_=rd[:])
                dstap = x2acc[:, w * HC:(w + 1) * HC]
                if r == 0:
                    for h in range(HEADS):
                        nc.vector.tensor_scalar(
                            out=dstap[:, h * HID:(h + 1) * HID],
                            in0=pon[:, h * HID:(h + 1) * HID],
                            scalar1=rd[:, h:h + 1], scalar2=None,
                            op0=mybir.AluOpType.mult)
                else:
                    tmp = wp.tile([128, HC], BF16, tag="tmp")
                    for h in range(HEADS):
                        nc.scalar.mul(
                            out=tmp[:, h * HID:(h + 1) * HID],
                            in_=pon[:, h * HID:(h + 1) * HID],
                            mul=rd[:, h:h + 1])
                    nc.vector.tensor_tensor(
                        out=dstap, in0=dstap, in1=tmp[:],
                        op=mybir.AluOpType.add)

            # ---------------- layer-2 dense phase: h2 tables ----------------
            # table rows are in (core, slot) order so each 4-slot group is one
            # contiguous write
            def dense_groups(r):
                DB = 4
                thunks = []
                for co in range(NCORES):
                    for so0 in range(0, W, DB):
                        nb = min(DB, W - so0)

                        def run(co=co, so0=so0, nb=nb):
                            lhs4 = []
                            for k in range(2):
                                l4 = wp.tile([128, DB * 128], BF16,
                                             tag=f"lhs4{k}", bufs=2, name=f"l4{k}")
                                nc.sync.dma_start(
                                    out=l4[:, 0:nb * 128],
                                    in_=d_x2Tf[co * 256 + k * 128:
                                               co * 256 + (k + 1) * 128,
                                               so0 * 128:(so0 + nb) * 128])
                                lhs4.append(l4)
                            hsb4 = wp.tile([128, DB, MMN], BF16, tag="hsbd",
                                           bufs=2)
                            for i in range(nb):
                                ph = pp.tile([128, MMN], F32, space="PSUM",
                                             tag="ph")
                                for k in range(2):
                                    nc.tensor.matmul(
                                        out=ph[:],
                                        lhsT=lhs4[k][:, i * 128:(i + 1) * 128],
                                        rhs=t_w2e[r][k][:],
                                        start=(k == 0), stop=(k == 1))
                                nc.scalar.copy(out=hsb4[:, i, :], in_=ph[:])
                            base = (co * W + so0) * 128
                            nc.sync.dma_start(
                                out=tab2[r][base:base + nb * 128, 0:MMN]
                                    .rearrange("(k p) c -> p k c", p=128),
                                in_=hsb4[:, 0:nb, :])

                        thunks.append(run)
                return thunks

            # ---------------- layer-2 edge phase: gather ------------------
            def edge_phase2(r, dense_next):
                s = s2[r]
                KL, KH = s["KL"], s["KH"]
                chb, colL, colH = s["chbase"], s["colL"], s["colH"]
                t_idx = ei.tile([128, s["totcol"]], I16, tag="idx")
                nc.sync.dma_start(out=t_idx[:], in_=d_idx[r][:])

                def stage_a(w):
                    kl, kh = int(KL[w]), int(KH[w])
                    K = kl + kh
                    c0 = int(chb[w])
                    g = gp.tile([128, K2max, TABW], BF16, tag="gw", bufs=3)
                    for grp, (kk, coff, base) in enumerate(
                            ((kl, int(colL[w]), 0), (kh, int(colH[w]), kl))):
                        src_ap = tab2[r][:] if grp == 0 else tab2[r][SPLIT:NPAD, :]
                        for sub in range(0, kk, GMAX):
                            nk = min(GMAX, kk - sub)
                            nc.gpsimd.dma_gather(
                                g[:, base + sub:base + sub + nk, :], src_ap,
                                t_idx[:, coff + sub * 8:coff + (sub + nk) * 8],
                                nk * 128, nk * 128, TABW,
                                queue_num=qn[0] % 4)
                            qn[0] += 1
                    ohT = gp.tile([128, K2max, 128], BF16, tag="ohT",
                                  bufs=2, name="ohT")
                    nc.sync.dma_start(
                        out=ohT[:, 0:K, :].rearrange("p k d -> p (k d)"),
                        in_=d_ohT[r][:, c0 * 128:(c0 + K) * 128])
                    # al_d for this window's own 128 dst rows
                    x2w = wp.tile([128, 2, 128], BF16, tag="x2w", bufs=2)
                    nc.sync.dma_start(
                        out=x2w[:],
                        in_=d_x2oT[:, w * 128:(w + 1) * 128]
                            .rearrange("(k p) n -> p k n", p=128))
                    paw = pp.tile([128, HEADS], F32, space="PSUM", tag="pT")
                    for k in range(2):
                        nc.tensor.matmul(
                            out=paw[:], lhsT=x2w[:, k, :],
                            rhs=t_w2ad[r][k][:],
                            start=(k == 0), stop=(k == 1))
                    aw = wp.tile([128, HEADS], BF16, tag="aw", bufs=2)
                    nc.vector.tensor_copy(out=aw[:], in_=paw[:])
                    return (w, K, c0, g, ohT, aw)

                def stage_b(st):
                    w, K, c0, g, ohT, aw = st
                    t_oh = gp.tile([128, K2max, 128], BF16, tag="ohL",
                                   bufs=2, name="ohL")
                    nc.sync.dma_start(
                        out=t_oh[:, 0:K, :].rearrange("p k d -> p (k d)"),
                        in_=d_oh2[r][:, c0 * 128:(c0 + K) * 128])
                    pz = pp.tile([128, K2max, HEADS], F32, space="PSUM",
                                 tag="pald")
                    for c in range(K):
                        nc.tensor.matmul(out=pz[:, c, :], lhsT=ohT[:, c, :],
                                         rhs=aw[:], start=True, stop=True,
                                         skip_group_check=True)
                    zl = wp.tile([128, K2max, HEADS], BF16, tag="zl", bufs=2)
                    nc.vector.tensor_tensor(
                        out=zl[:, 0:K, :], in0=g[:, 0:K, HC:MMN],
                        in1=pz[:, 0:K, :], op=mybir.AluOpType.add)
                    nc.vector.scalar_tensor_tensor(
                        out=zl[:, 0:K, :], in0=zl[:, 0:K, :], scalar=0.2,
                        in1=zl[:, 0:K, :],
                        op0=mybir.AluOpType.mult, op1=mybir.AluOpType.max)
                    e1 = wp.tile([128, K2max, HEADS], F32, tag="e1", bufs=2)
                    nc.scalar.activation(out=e1[:, 0:K, :], in_=zl[:, 0:K, :],
                                         func=Exp)
                    nc.scalar.copy(out=g[:, 0:K, HC:MMN], in_=e1[:, 0:K, :])
                    for h in range(HEADS):
                        nc.vector.tensor_tensor(
                            out=g[:, 0:K, h * HID:(h + 1) * HID],
                            in0=g[:, 0:K, h * HID:(h + 1) * HID],
                            in1=e1[:, 0:K, h:h + 1].to_broadcast([128, K, HID]),
                            op=mybir.AluOpType.mult)
                    return (w, K, t_oh, g)

                def stage_c(st):
                    w, K, t_oh, g = st
                    po = pp.tile([128, MMN], F32, space="PSUM", tag="po")
                    for c in range(K):
                        nc.tensor.matmul(out=po[:], lhsT=t_oh[:, c, :],
                                         rhs=g[:, c, 0:MMN],
                                         start=(c == 0), stop=(c == K - 1))
                    _epilogue2(r, w, po)

                nd = len(dense_next)
                emitted = 0
                states = {}
                for w in range(W):
                    states[w] = stage_a(w)
                    want = (w + 1) * nd // W
                    while emitted < want:
                        dense_next[emitted]()
                        emitted += 1
                    if w >= 1:
                        states[w - 1] = stage_b(states[w - 1])
                    if w >= 2:
                        stage_c(states.pop(w - 2))
                states[W - 1] = stage_b(states[W - 1])
                stage_c(states.pop(W - 2))
                stage_c(states.pop(W - 1))

            def _epilogue2(r, w, po):
                pon = wp.tile([128, HC], BF16, tag="pon", bufs=3)
                nc.vector.tensor_copy(out=pon[:], in_=po[:, 0:HC])
                rd = wp.tile([128, HEADS], F32, tag="rd", bufs=3)
                nc.vector.tensor_scalar(
                    out=rd[:], in0=po[:, HC:MMN], scalar1=1e-16,
                    scalar2=None, op0=mybir.AluOpType.add)
                nc.vector.reciprocal(out=rd[:], in_=rd[:])
                nc.vector.tensor_scalar(
                    out=rd[:], in0=rd[:], scalar1=0.25, scalar2=None,
                    op0=mybir.AluOpType.mult)
                dstap = x3acc[:, w * HID:(w + 1) * HID]
                for h in range(HEADS):
                    if r == 0 and h == 0:
                        nc.vector.tensor_scalar(
                            out=dstap, in0=pon[:, 0:HID],
                            scalar1=rd[:, 0:1], scalar2=None,
                            op0=mybir.AluOpType.mult)
                    else:
                        t64 = wp.tile([128, HID], BF16, tag="t64")
                        nc.scalar.mul(out=t64[:], in_=pon[:, h * HID:(h + 1) * HID],
                                      mul=rd[:, h:h + 1])
                        nc.vector.tensor_tensor(
                            out=dstap, in0=dstap, in1=t64[:],
                            op=mybir.AluOpType.add)

            # ================= layer 1 =================
            for r in range(3):
                edge_phase1(r)

            # ---- x2 = relu(acc + b1), transpose, store, AllGather
            for w in range(W):
                y = wp.tile([128, HC], BF16, tag="y")
                nc.vector.tensor_tensor(out=y[:], in0=x2acc[:, w * HC:(w + 1) * HC],
                                        in1=t_b1b[:], op=mybir.AluOpType.add)
                nc.vector.tensor_scalar(out=y[:], in0=y[:], scalar1=0.0,
                                        scalar2=None, op0=mybir.AluOpType.max)
                y2T = wp.tile([128, 2, 128], BF16, tag="x2w", bufs=2)
                for k in range(2):
                    psT = pp.tile([128, 128], BF16, space="PSUM", tag="pT")
                    nc.tensor.transpose(out=psT[:], in_=y[:, k * 128:(k + 1) * 128],
                                        identity=t_identb[:])
                    nc.scalar.copy(out=y2T[:, k, :], in_=psT[:])
                    nc.sync.dma_start(
                        out=d_x2oT[k * 128:(k + 1) * 128, w * 128:(w + 1) * 128],
                        in_=y2T[:, k, :])
            nc.gpsimd.collective_compute(
                "AllGather", mybir.AluOpType.bypass,
                replica_groups=[list(range(NCORES))],
                ins=[d_x2oT[:]], outs=[d_x2Tf[:]])

            # ================= layer 2 =================
            for t in dense_groups(0):
                t()
            for r in range(3):
                nxt = dense_groups(r + 1) if r < 2 else []
                edge_phase2(r, nxt)

            # ---- final: out = relu(x3 + b2) @ Wl + bl
            for w in range(W):
                y = wp.tile([128, HID], BF16, tag="t64")
                nc.vector.tensor_tensor(out=y[:], in0=x3acc[:, w * HID:(w + 1) * HID],
                                        in1=t_b2b[:], op=mybir.AluOpType.add)
                nc.vector.tensor_scalar(out=y[:], in0=y[:], scalar1=0.0,
                                        scalar2=None, op0=mybir.AluOpType.max)
                psT = pp.tile([128, 128], BF16, space="PSUM", tag="pT")
                nc.tensor.transpose(out=psT[:HID, :], in_=y[:], identity=t_identb[:])
                x3T = wp.tile([HID, 128], BF16, tag="x3T")
                nc.scalar.copy(out=x3T[:], in_=psT[:HID, :])
                pf = pp.tile([128, HID], F32, space="PSUM", tag="po")
                nc.tensor.matmul(out=pf[:], lhsT=x3T[:], rhs=t_wl[:],
                                 start=True, stop=True)
                fo = wp.tile([128, HID], F32, tag="fo")
                nc.vector.tensor_tensor(out=fo[:], in0=pf[:], in1=t_blb[:],
                                        op=mybir.AluOpType.add)
                nc.sync.dma_start(out=d_out[w * 128:(w + 1) * 128, :], in_=fo[:])

    nc.compile()
    return nc


# ------------------------------------------------------------------- driver

def kernel(x, e_blocks, e_spatial, e_similar, W1, a1s, a1d, b1,
           W2, a2s, a2d, b2, Wl, bl, **_ignored):
    global last_results
    x = np.asarray(x, np.float32)
    W1 = np.asarray(W1, np.float32)
    a1s = np.asarray(a1s, np.float32)
    a1d = np.asarray(a1d, np.float32)
    b1 = np.asarray(b1, np.float32)
    W2 = np.asarray(W2, np.float32)
    a2s = np.asarray(a2s, np.float32)
    a2d = np.asarray(a2d, np.float32)
    b2 = np.asarray(b2, np.float32)
    Wl = np.asarray(Wl, np.float32)
    bl = np.asarray(bl, np.float32)

    loops = np.arange(N, dtype=np.int64)
    edge_sets = []
    for ei, add_loops in ((e_blocks, False), (e_spatial, True), (e_similar, True)):
        src = np.asarray(ei[0], np.int64)
        dst = np.asarray(ei[1], np.int64)
        if add_loops:
            src = np.concatenate([src, loops])
            dst = np.concatenate([dst, loops])
        edge_sets.append((src, dst))

    win_of, core_of, slot_of = _assign_windows(edge_sets)

    xTb = _bf16(np.concatenate([x.T, np.zeros((IN_CH, NPAD - N), np.float32)],
                               axis=1))
    s1, cores1 = [], []
    s2, cores2 = [], []
    for src, dst in edge_sets:
        sch1, pc1 = _prep_edges_l1(src, dst, core_of, slot_of, xTb)
        s1.append(sch1)
        cores1.append(pc1)
        sch2, pc2 = _prep_edges_l2(src, dst, core_of, slot_of)
        s2.append(sch2)
        cores2.append(pc2)

    w1e = np.zeros((3, IN_CH, MMN), np.float32)
    w1ad = np.zeros((3, IN_CH, HEADS), np.float32)
    w2e = np.zeros((3, 2 * 128, MMN), np.float32)
    w2ad = np.zeros((3, 2 * 128, HEADS), np.float32)
    for r in range(3):
        w1e[r, :, :HC] = W1[r]
        w1e[r, :, HC:MMN] = W1[r] @ _block_diag_a(a1s[r])
        w1ad[r] = W1[r] @ _block_diag_a(a1d[r])
        w2e[r, :, :HC] = W2[r]
        w2e[r, :, HC:MMN] = W2[r] @ _block_diag_a(a2s[r])
        w2ad[r] = W2[r] @ _block_diag_a(a2d[r])
    w2e = w2e.reshape(3, 2, 128, MMN)
    w2ad = w2ad.reshape(3, 2, 128, HEADS)

    common = {
        "w1e": _bf16(w1e), "w1ad": _bf16(w1ad),
        "w2e": _bf16(w2e), "w2ad": _bf16(w2ad),
        "b1b": _bf16(np.tile(b1.sum(0)[None, :], (128, 1))),
        "b2b": _bf16(np.tile(b2.sum(0)[None, :], (128, 1))),
        "blb": np.tile(bl[None, :], (128, 1)).astype(np.float32),
        "wl": _bf16(Wl),
        "identb": _bf16(np.eye(128, dtype=np.float32)),
    }
    in_maps = []
    for c in range(NCORES):
        m = dict(common)
        for r in range(3):
            xs, xd, oh1 = cores1[r][c]
            m[f"xs{r}"] = xs
            m[f"xd{r}"] = xd
            m[f"oh1_{r}"] = oh1
            ia, oh2, ohT = cores2[r][c]
            m[f"idx{r}"] = ia
            m[f"oh2_{r}"] = oh2
            m[f"ohT{r}"] = ohT
        in_maps.append(m)

    nc = _build(s1, s2, win_of)
    res = run_bass_kernel_spmd(nc, in_maps, core_ids=list(range(NCORES)))
    last_results = res
    full = np.zeros((NPAD, HID), np.float32)
    for c in range(NCORES):
        oc = res.results[c]["out"]
        for s_ in range(W):
            w = int(win_of[c, s_])
            full[w * 128:(w + 1) * 128] = oc[s_ * 128:(s_ + 1) * 128]
    return full[:N].astype(np.float32)


# revision 28
# speedup vs baseline: 4.7582x; 1.0728x over previous
"""HeteroGAT (3-relation, 2-layer GAT + linear head) on 8 Trainium2 cores.

v3: Layer 1 is gather-free: the host pre-permutes x rows into per-edge
order (src and dst streams, f-major), streamed contiguously via HWDGE;
per-chunk projection matmuls produce h|al_s and accumulate al_d into the
same PSUM columns. Layer 2 keeps per-relation h2 tables in HBM with
dma_gather (768B rows), now 3-deep buffered. One-hots are built with
full-rate tensor_scalar is_equal ops (oh) and partition-broadcast
is_equal (ohT); exp(leaky(z)) = max(exp(z), exp(0.2z)) via two ACT ops.
Scatter is a single 260-wide matmul per 128-edge chunk with the softmax
denominators in columns 256:260. Only cross-core exchange: AllGather of
transposed layer-1 activations (bf16).
"""

import numpy as np

import concourse.bacc as bacc
import concourse.bass as bass
import concourse.mybir as mybir
import concourse.tile as tile
from concourse.bass_utils import run_bass_kernel_spmd

F32 = mybir.dt.float32
BF16 = mybir.dt.bfloat16
I16 = mybir.dt.int16

N = 50000
NPAD = 50176            # 392 * 128
NCORES = 8
NOWN = 6272             # 49 * 128 rows per core
W = 49                  # window slots per core
NT = NPAD // 128        # 392 global node tiles / windows
SPLIT = 32768           # int16 index limit for dma_gather
TABW = 384              # L2 table row stride in bf16 elems (768 B, %256)
HC = 256                # feature columns
MMN = 260               # 256 feats + 4 attention cols
IN_CH = 128
HID = 64
HEADS = 4
GMAX = 8                # chunks per dma_gather call (1024 idx is the ucode limit)

last_results = None


# ----------------------------------------------------------------- host prep

def _bf16(a):
    import ml_dtypes
    return np.asarray(a, np.float32).astype(ml_dtypes.bfloat16)


def _assign_windows(edge_sets):
    """Snake-assign the 392 global windows to (core, slot) by total edge
    count. Returns win_of [NCORES, W] and core_of/slot_of [NT]."""
    score = np.zeros(NT, np.int64)
    for src, dst in edge_sets:
        score += np.bincount(dst >> 7, minlength=NT)
    order = np.argsort(-score, kind="stable")
    win_of = np.zeros((NCORES, W), np.int64)
    for s in range(W):
        grp = order[s * NCORES:(s + 1) * NCORES]
        if s % 2:
            grp = grp[::-1]
        for c in range(NCORES):
            win_of[c, s] = grp[c]
    core_of = np.zeros(NT, np.int64)
    slot_of = np.zeros(NT, np.int64)
    for c in range(NCORES):
        for s in range(W):
            core_of[win_of[c, s]] = c
            slot_of[win_of[c, s]] = s
    return win_of, core_of, slot_of


def _prep_edges_l1(src, dst, core_of, slot_of, xTb):
    """Layer-1 host pre-gather. Edges bucketed by (core, slot); chunk counts
    maxed over cores so the device schedule is common. Returns
    (K1 [W], per_core list of (xs [128, totch*128] bf16,
    xd [...], dl [128, totch] bf16))."""
    E = src.size
    wg = dst >> 7
    sv = slot_of[wg]
    cv = core_of[wg]
    okey = cv * W + sv
    order = np.argsort(okey, kind="stable")
    so, ss, sd = okey[order], src[order], dst[order]
    cnts = np.bincount(okey, minlength=NCORES * W)
    seg_start = np.concatenate([[0], np.cumsum(cnts)[:-1]])
    pos = np.arange(E) - seg_start[so]

    mx = cnts.reshape(NCORES, W).max(axis=0)
    K1 = np.maximum((mx + 127) // 128, 1)
    chbase = np.concatenate([[0], np.cumsum(K1)[:-1]])
    totch = int(K1.sum())

    svs, cvs = sv[order], cv[order]
    col = (chbase[svs] + (pos >> 7)) * 128 + (pos & 127)

    eye_ext = np.zeros((129, 128), np.float32)
    eye_ext[1:] = np.eye(128, dtype=np.float32)
    eye_ext = _bf16(eye_ext)
    per_core = []
    for c in range(NCORES):
        m = cvs == c
        srccols = np.zeros(totch * 128, np.int64)
        dstcols = np.zeros(totch * 128, np.int64)
        dlv = np.full(totch * 128, -1, np.int64)
        srccols[col[m]] = ss[m]
        dstcols[col[m]] = sd[m]
        dlv[col[m]] = sd[m] & 127
        xs = np.ascontiguousarray(xTb[:, srccols])
        xd = np.ascontiguousarray(xTb[:, dstcols])
        oh_rows = eye_ext[dlv + 1].reshape(totch, 128, 128)
        oh = np.ascontiguousarray(
            oh_rows.transpose(1, 0, 2).reshape(128, totch * 128))
        per_core.append((xs, xd, oh))
    return dict(K1=K1, chbase=chbase, totch=totch), per_core


def _prep_edges_l2(src, dst, core_of, slot_of):
    """Layer-2 gather schedule (same as baseline) + dlT row for ohT builds.
    Returns (sched, per_core list of (idx [128, totcol] i16,
    dl [128, totch] bf16, dlT [1, totch*128] bf16))."""
    E = src.size
    wg = dst >> 7
    sv = slot_of[wg]
    cv = core_of[wg]
    # gather table rows live in (core, slot) order
    swg = src >> 7
    srcpos = (core_of[swg] * W + slot_of[swg]) * 128 + (src & 127)
    gv = (srcpos >= SPLIT).astype(np.int64)
    okey = cv * (2 * W) + sv * 2 + gv
    order = np.argsort(okey, kind="stable")
    so, ss, sd = okey[order], src[order], dst[order]
    cnts = np.bincount(okey, minlength=NCORES * 2 * W)
    seg_start = np.concatenate([[0], np.cumsum(cnts)[:-1]])
    pos = np.arange(E) - seg_start[so]

    mx = cnts.reshape(NCORES, 2 * W).max(axis=0)
    Kg = (mx + 127) // 128
    KL, KH = Kg[0::2].copy(), Kg[1::2].copy()
    KL[(KL + KH) == 0] = 1
    Ksum = KL + KH
    chbase = np.concatenate([[0], np.cumsum(Ksum)[:-1]])
    totch = int(Ksum.sum())
    colL = np.zeros(W, np.int64)
    colH = np.zeros(W, np.int64)
    cum = 0
    for w in range(W):
        colL[w] = cum
        cum += KL[w] * 8
        colH[w] = cum
        cum += KH[w] * 8
    totcol = int(cum)

    svs, gvs, cvs = sv[order], gv[order], cv[order]
    j = np.where(gvs == 0, pos, KL[svs] * 128 + pos)
    ch = chbase[svs] + (j >> 7)
    prow = j & 127
    colbase = np.where(gvs == 0, colL[svs], colH[svs])
    icol = colbase + (pos >> 4)
    irow = pos & 15
    sp_s = srcpos[order]
    idxval = np.where(gvs == 0, sp_s, sp_s - SPLIT).astype(np.int16)
    dloc = (sd & 127).astype(np.float32)

    eye_ext = np.zeros((129, 128), np.float32)
    eye_ext[1:] = np.eye(128, dtype=np.float32)
    eye_ext = _bf16(eye_ext)
    per_core = []
    for c in range(NCORES):
        m = cvs == c
        ia = np.zeros((16, totcol), np.int16)
        ia[irow[m], icol[m]] = idxval[m]
        dlv = np.full(totch * 128, -1, np.int64)
        dlv[ch[m] * 128 + prow[m]] = sd[m] & 127
        oh_rows = eye_ext[dlv + 1].reshape(totch, 128, 128)
        oh = np.ascontiguousarray(
            oh_rows.transpose(1, 0, 2).reshape(128, totch * 128))
        ohT = np.ascontiguousarray(
            oh_rows.transpose(2, 0, 1).reshape(128, totch * 128))
        per_core.append((np.ascontiguousarray(np.tile(ia, (8, 1))),
                         oh, ohT))
    sched = dict(KL=KL, KH=KH, chbase=chbase, colL=colL, colH=colH,
                 totch=totch, totcol=totcol)
    return sched, per_core


def _block_diag_a(a):            # a: [4, 64] -> [256, 4]
    A = np.zeros((HEADS * HID, HEADS), np.float32)
    A[np.arange(HEADS * HID), np.arange(HEADS * HID) // HID] = a.reshape(-1)
    return A


# ------------------------------------------------------------ device program

def _build(s1, s2, win_of):
    nc = bacc.Bacc("TRN2", num_devices=NCORES, num_swdge_queues=4)
    Exp = mybir.ActivationFunctionType.Exp

    d_xs, d_xd, d_oh1 = [], [], []
    for r in range(3):
        t1 = s1[r]["totch"]
        d_xs.append(nc.dram_tensor(f"xs{r}", [128, t1 * 128], BF16,
                                   kind="ExternalInput"))
        d_xd.append(nc.dram_tensor(f"xd{r}", [128, t1 * 128], BF16,
                                   kind="ExternalInput"))
        d_oh1.append(nc.dram_tensor(f"oh1_{r}", [128, t1 * 128], BF16,
                                    kind="ExternalInput"))
    d_idx, d_oh2, d_ohT = [], [], []
    for r in range(3):
        s = s2[r]
        d_idx.append(nc.dram_tensor(f"idx{r}", [128, s["totcol"]], I16,
                                    kind="ExternalInput"))
        d_oh2.append(nc.dram_tensor(f"oh2_{r}", [128, s["totch"] * 128], BF16,
                                    kind="ExternalInput"))
        d_ohT.append(nc.dram_tensor(f"ohT{r}", [128, s["totch"] * 128], BF16,
                                    kind="ExternalInput"))
    d_w1e = nc.dram_tensor("w1e", [3, IN_CH, MMN], BF16, kind="ExternalInput")
    d_w1ad = nc.dram_tensor("w1ad", [3, IN_CH, HEADS], BF16, kind="ExternalInput")
    d_w2e = nc.dram_tensor("w2e", [3, 2, 128, MMN], BF16, kind="ExternalInput")
    d_w2ad = nc.dram_tensor("w2ad", [3, 2, 128, HEADS], BF16, kind="ExternalInput")
    d_b1b = nc.dram_tensor("b1b", [128, HC], BF16, kind="ExternalInput")
    d_b2b = nc.dram_tensor("b2b", [128, HID], BF16, kind="ExternalInput")
    d_blb = nc.dram_tensor("blb", [128, HID], F32, kind="ExternalInput")
    d_wl = nc.dram_tensor("wl", [HID, HID], BF16, kind="ExternalInput")
    d_identb = nc.dram_tensor("identb", [128, 128], BF16, kind="ExternalInput")

    tab2 = [nc.dram_tensor(f"tab2_{r}", [NPAD, TABW], BF16, kind="Internal")
            for r in range(3)]
    d_x2oT = nc.dram_tensor("x2oT", [2 * 128, NOWN], BF16, kind="Internal")
    d_x2Tf = nc.dram_tensor("x2Tf", [NCORES * 2 * 128, NOWN], BF16,
                            kind="Internal", addr_space="Shared")
    d_out = nc.dram_tensor("out", [NOWN, HID], F32, kind="ExternalOutput")

    qn = [0]    # rotating SWDGE queue

    with tile.TileContext(nc) as tc:
        with (
            tc.tile_pool(name="const", bufs=1) as cp,
            tc.tile_pool(name="acc", bufs=1) as ac,
            tc.tile_pool(name="eio", bufs=2) as ei,
            tc.tile_pool(name="gw", bufs=2) as gp,
            tc.tile_pool(name="st", bufs=2) as sp,
            tc.tile_pool(name="work", bufs=2) as wp,
            tc.tile_pool(name="ps", bufs=2, space="PSUM") as pp,
        ):
            # ---- constants
            t_identb = cp.tile([128, 128], BF16)
            nc.sync.dma_start(out=t_identb[:], in_=d_identb[:])
            t_w1e = [cp.tile([IN_CH, MMN], BF16, tag=f"w1e{r}", name=f"w1e{r}")
                     for r in range(3)]
            t_w1ad = [cp.tile([IN_CH, HEADS], BF16, tag=f"w1ad{r}", name=f"w1ad{r}")
                      for r in range(3)]
            for r in range(3):
                nc.sync.dma_start(out=t_w1e[r][:], in_=d_w1e[r])
                nc.sync.dma_start(out=t_w1ad[r][:], in_=d_w1ad[r])
            t_w2e = [[cp.tile([128, MMN], BF16, tag=f"w2e{r}{k}", name=f"w2e{r}{k}")
                      for k in range(2)] for r in range(3)]
            t_w2ad = [[cp.tile([128, HEADS], BF16, tag=f"w2ad{r}{k}",
                               name=f"w2ad{r}{k}") for k in range(2)]
                      for r in range(3)]
            for r in range(3):
                for k in range(2):
                    nc.sync.dma_start(out=t_w2e[r][k][:], in_=d_w2e[r, k])
                    nc.sync.dma_start(out=t_w2ad[r][k][:], in_=d_w2ad[r, k])
            t_b1b = cp.tile([128, HC], BF16)
            nc.sync.dma_start(out=t_b1b[:], in_=d_b1b[:])
            t_b2b = cp.tile([128, HID], BF16)
            nc.sync.dma_start(out=t_b2b[:], in_=d_b2b[:])
            t_blb = cp.tile([128, HID], F32)
            nc.sync.dma_start(out=t_blb[:], in_=d_blb[:])
            t_wl = cp.tile([HID, HID], BF16)
            nc.sync.dma_start(out=t_wl[:], in_=d_wl[:])
            x2acc = ac.tile([128, W * HC], BF16)
            x3acc = ac.tile([128, W * HID], BF16)

            K1max = max(int(s1[r]["K1"].max()) for r in range(3))
            K2max = max(int((s2[r]["KL"] + s2[r]["KH"]).max()) for r in range(3))

            # ---------------- layer-1 edge phase: stream + project ----------
            def edge_phase1(r):
                K1, chb = s1[r]["K1"], s1[r]["chbase"]

                def stage_a(w):
                    K = int(K1[w])
                    c0 = int(chb[w])
                    xs = sp.tile([128, K1max * 128], BF16, tag="xs", name="xs")
                    nc.sync.dma_start(
                        out=xs[:, 0:K * 128],
                        in_=d_xs[r][:, c0 * 128:(c0 + K) * 128])
                    xd = sp.tile([128, K1max * 128], BF16, tag="xd", name="xd")
                    nc.sync.dma_start(
                        out=xd[:, 0:K * 128],
                        in_=d_xd[r][:, c0 * 128:(c0 + K) * 128])
                    hall = wp.tile([128, K1max, MMN], BF16, tag="hall", bufs=3)
                    for c in range(K):
                        ph = pp.tile([128, MMN], F32, space="PSUM", tag="ph")
                        nc.tensor.matmul(out=ph[:], lhsT=xs[:, c * 128:(c + 1) * 128],
                                         rhs=t_w1e[r][:], start=True, stop=False,
                                         skip_group_check=True)
                        nc.tensor.matmul(out=ph[:, HC:MMN],
                                         lhsT=xd[:, c * 128:(c + 1) * 128],
                                         rhs=t_w1ad[r][:], start=False, stop=True,
                                         skip_group_check=True)
                        nc.scalar.copy(out=hall[:, c, :], in_=ph[:])
                    return (w, K, c0, hall)

                def stage_b(st):
                    w, K, c0, hall = st
                    oh1 = sp.tile([128, K1max, 128], BF16, tag="oh1", name="oh1")
                    nc.sync.dma_start(
                        out=oh1[:, 0:K, :].rearrange("p k d -> p (k d)"),
                        in_=d_oh1[r][:, c0 * 128:(c0 + K) * 128])
                    # ex = exp(leaky(z)) = exp(max(z, 0.2 z)), written into the
                    # denominator columns of the scatter rhs
                    zl = wp.tile([128, K1max, HEADS], BF16, tag="zl", bufs=2)
                    nc.vector.scalar_tensor_tensor(
                        out=zl[:, 0:K, :], in0=hall[:, 0:K, HC:MMN], scalar=0.2,
                        in1=hall[:, 0:K, HC:MMN],
                        op0=mybir.AluOpType.mult, op1=mybir.AluOpType.max)
                    nc.scalar.activation(out=hall[:, 0:K, HC:MMN],
                                         in_=zl[:, 0:K, :], func=Exp)
                    for h in range(HEADS):
                        nc.vector.tensor_tensor(
                            out=hall[:, 0:K, h * HID:(h + 1) * HID],
                            in0=hall[:, 0:K, h * HID:(h + 1) * HID],
                            in1=hall[:, 0:K, HC + h:HC + h + 1]
                                .to_broadcast([128, K, HID]),
                            op=mybir.AluOpType.mult)
                    return (w, K, oh1, hall)

                def stage_c(st):
                    w, K, oh1, hall = st
                    po = pp.tile([128, MMN], F32, space="PSUM", tag="po")
                    for c in range(K):
                        nc.tensor.matmul(out=po[:], lhsT=oh1[:, c, :],
                                         rhs=hall[:, c, :],
                                         start=(c == 0), stop=(c == K - 1))
                    _epilogue1(r, w, po)

                states = {}
                for w in range(W):
                    states[w] = stage_a(w)
                    if w >= 1:
                        states[w - 1] = stage_b(states[w - 1])
                    if w >= 2:
                        stage_c(states.pop(w - 2))
                states[W - 1] = stage_b(states[W - 1])
                stage_c(states.pop(W - 2))
                stage_c(states.pop(W - 1))

            def _epilogue1(r, w, po):
                pon = wp.tile([128, HC], BF16, tag="pon", bufs=3)
                nc.vector.tensor_copy(out=pon[:], in_=po[:, 0:HC])
                rd = wp.tile([128, HEADS], F32, tag="rd", bufs=3)
                nc.vector.tensor_scalar(
                    out=rd[:], in0=po[:, HC:MMN], scalar1=1e-16,
                    scalar2=None, op0=mybir.AluOpType.add)
                nc.vector.reciprocal(out=rd[:], in_=rd[:])
                dstap = x2acc[:, w * HC:(w + 1) * HC]
                if r == 0:
                    for h in range(HEADS):
                        nc.vector.tensor_scalar(
                            out=dstap[:, h * HID:(h + 1) * HID],
                            in0=pon[:, h * HID:(h + 1) * HID],
                            scalar1=rd[:, h:h + 1], scalar2=None,
                            op0=mybir.AluOpType.mult)
                else:
                    tmp = wp.tile([128, HC], BF16, tag="tmp")
                    for h in range(HEADS):
                        nc.scalar.mul(
                            out=tmp[:, h * HID:(h + 1) * HID],
                            in_=pon[:, h * HID:(h + 1) * HID],
                            mul=rd[:, h:h + 1])
                    nc.vector.tensor_tensor(
                        out=dstap, in0=dstap, in1=tmp[:],
                        op=mybir.AluOpType.add)

            # ---------------- layer-2 dense phase: h2 tables ----------------
            # table rows are in (core, slot) order so each 4-slot group is one
            # contiguous write
            def dense_groups(r):
                DB = 4
                thunks = []
                for co in range(NCORES):
                    for so0 in range(0, W, DB):
                        nb = min(DB, W - so0)

                        def run(co=co, so0=so0, nb=nb):
                            lhs4 = []
                            for k in range(2):
                                l4 = wp.tile([128, DB * 128], BF16,
                                             tag=f"lhs4{k}", bufs=2, name=f"l4{k}")
                                nc.sync.dma_start(
                                    out=l4[:, 0:nb * 128],
                                    in_=d_x2Tf[co * 256 + k * 128:
                                               co * 256 + (k + 1) * 128,
                                               so0 * 128:(so0 + nb) * 128])
                                lhs4.append(l4)
                            hsb4 = wp.tile([128, DB, MMN], BF16, tag="hsbd",
                                           bufs=2)
                            for i in range(nb):
                                ph = pp.tile([128, MMN], F32, space="PSUM",
                                             tag="ph")
                                for k in range(2):
                                    nc.tensor.matmul(
                                        out=ph[:],
                                        lhsT=lhs4[k][:, i * 128:(i + 1) * 128],
                                        rhs=t_w2e[r][k][:],
                                        start=(k == 0), stop=(k == 1))
                                nc.scalar.copy(out=hsb4[:, i, :], in_=ph[:])
                            base = (co * W + so0) * 128
                            nc.sync.dma_start(
                                out=tab2[r][base:base + nb * 128, 0:MMN]
                                    .rearrange("(k p) c -> p k c", p=128),
                                in_=hsb4[:, 0:nb, :])

                        thunks.append(run)
                return thunks

            # ---------------- layer-2 edge phase: gather ------------------
            def edge_phase2(r, dense_next):
                s = s2[r]
                KL, KH = s["KL"], s["KH"]
                chb, colL, colH = s["chbase"], s["colL"], s["colH"]
                t_idx = ei.tile([128, s["totcol"]], I16, tag="idx")
                nc.sync.dma_start(out=t_idx[:], in_=d_idx[r][:])

                def stage_a(w):
                    kl, kh = int(KL[w]), int(KH[w])
                    K = kl + kh
                    c0 = int(chb[w])
                    g = gp.tile([128, K2max, TABW], BF16, tag="gw", bufs=3)
                    for grp, (kk, coff, base) in enumerate(
                            ((kl, int(colL[w]), 0), (kh, int(colH[w]), kl))):
                        src_ap = tab2[r][:] if grp == 0 else tab2[r][SPLIT:NPAD, :]
                        for sub in range(0, kk, GMAX):
                            nk = min(GMAX, kk - sub)
                            nc.gpsimd.dma_gather(
                                g[:, base + sub:base + sub + nk, :], src_ap,
                                t_idx[:, coff + sub * 8:coff + (sub + nk) * 8],
                                nk * 128, nk * 128, TABW,
                                queue_num=qn[0] % 4)
                            qn[0] += 1
                    ohT = gp.tile([128, K2max, 128], BF16, tag="ohT",
                                  bufs=2, name="ohT")
                    nc.sync.dma_start(
                        out=ohT[:, 0:K, :].rearrange("p k d -> p (k d)"),
                        in_=d_ohT[r][:, c0 * 128:(c0 + K) * 128])
                    # al_d for this window's own 128 dst rows
                    x2w = wp.tile([128, 2, 128], BF16, tag="x2w", bufs=2)
                    nc.sync.dma_start(
                        out=x2w[:],
                        in_=d_x2oT[:, w * 128:(w + 1) * 128]
                            .rearrange("(k p) n -> p k n", p=128))
                    paw = pp.tile([128, HEADS], F32, space="PSUM", tag="pT")
                    for k in range(2):
                        nc.tensor.matmul(
                            out=paw[:], lhsT=x2w[:, k, :],
                            rhs=t_w2ad[r][k][:],
                            start=(k == 0), stop=(k == 1))
                    aw = wp.tile([128, HEADS], BF16, tag="aw", bufs=2)
                    nc.vector.tensor_copy(out=aw[:], in_=paw[:])
                    return (w, K, c0, g, ohT, aw)

                def stage_b(st):
                    w, K, c0, g, ohT, aw = st
                    t_oh = gp.tile([128, K2max, 128], BF16, tag="ohL",
                                   bufs=2, name="ohL")
                    nc.sync.dma_start(
                        out=t_oh[:, 0:K, :].rearrange("p k d -> p (k d)"),
                        in_=d_oh2[r][:, c0 * 128:(c0 + K) * 128])
                    pz = pp.tile([128, K2max, HEADS], F32, space="PSUM",
                                 tag="pald")
                    for c in range(K):
                        nc.tensor.matmul(out=pz[:, c, :], lhsT=ohT[:, c, :],
                                         rhs=aw[:], start=True, stop=True,
                                         skip_group_check=True)
                    zl = wp.tile([128, K2max, HEADS], BF16, tag="zl", bufs=2)
                    nc.vector.tensor_tensor(
                        out=zl[:, 0:K, :], in0=g[:, 0:K, HC:MMN],
                        in1=pz[:, 0:K, :], op=mybir.AluOpType.add)
                    nc.vector.scalar_tensor_tensor(
                        out=zl[:, 0:K, :], in0=zl[:, 0:K, :], scalar=0.2,
                        in1=zl[:, 0:K, :],
                        op0=mybir.AluOpType.mult, op1=mybir.AluOpType.max)
                    nc.scalar.activation(out=g[:, 0:K, HC:MMN],
                                         in_=zl[:, 0:K, :], func=Exp)
                    for h in range(HEADS):
                        nc.vector.tensor_tensor(
                            out=g[:, 0:K, h * HID:(h + 1) * HID],
                            in0=g[:, 0:K, h * HID:(h + 1) * HID],
                            in1=g[:, 0:K, HC + h:HC + h + 1]
                                .to_broadcast([128, K, HID]),
                            op=mybir.AluOpType.mult)
                    return (w, K, t_oh, g)

                def stage_c(st):
                    w, K, t_oh, g = st
                    po = pp.tile([128, MMN], F32, space="PSUM", tag="po")
                    for c in range(K):
                        nc.tensor.matmul(out=po[:], lhsT=t_oh[:, c, :],
                                         rhs=g[:, c, 0:MMN],
                                         start=(c == 0), stop=(c == K - 1))
                    _epilogue2(r, w, po)

                nd = len(dense_next)
                emitted = 0
                states = {}
                for w in range(W):
                    states[w] = stage_a(w)
                    want = (w + 1) * nd // W
                    while emitted < want:
                        dense_next[emitted]()
                        emitted += 1
                    if w >= 1:
                        states[w - 1] = stage_b(states[w - 1])
                    if w >= 2:
                        stage_c(states.pop(w - 2))
                states[W - 1] = stage_b(states[W - 1])
                stage_c(states.pop(W - 2))
                stage_c(states.pop(W - 1))

            def _epilogue2(r, w, po):
                pon = wp.tile([128, HC], BF16, tag="pon", bufs=3)
                nc.vector.tensor_copy(out=pon[:], in_=po[:, 0:HC])
                rd = wp.tile([128, HEADS], F32, tag="rd", bufs=3)
                nc.vector.tensor_scalar(
                    out=rd[:], in0=po[:, HC:MMN], scalar1=1e-16,
                    scalar2=None, op0=mybir.AluOpType.add)
                nc.vector.reciprocal(out=rd[:], in_=rd[:])
                nc.vector.tensor_scalar(
                    out=rd[:], in0=rd[:], scalar1=0.25, scalar2=None,
                    op0=mybir.AluOpType.mult)
                dstap = x3acc[:, w * HID:(w + 1) * HID]
                for h in range(HEADS):
                    if r == 0 and h == 0:
                        nc.vector.tensor_scalar(
                            out=dstap, in0=pon[:, 0:HID],
                            scalar1=rd[:, 0:1], scalar2=None,
                            op0=mybir.AluOpType.mult)
                    else:
                        t64 = wp.tile([128, HID], BF16, tag="t64")
                        nc.scalar.mul(out=t64[:], in_=pon[:, h * HID:(h + 1) * HID],
                                      mul=rd[:, h:h + 1])
                        nc.vector.tensor_tensor(
                            out=dstap, in0=dstap, in1=t64[:],
                            op=mybir.AluOpType.add)

            # ================= layer 1 =================
            for r in range(3):
                edge_phase1(r)

            # ---- x2 = relu(acc + b1), transpose, store, AllGather
            for w in range(W):
                y = wp.tile([128, HC], BF16, tag="y")
                nc.vector.tensor_tensor(out=y[:], in0=x2acc[:, w * HC:(w + 1) * HC],
                                        in1=t_b1b[:], op=mybir.AluOpType.add)
                nc.vector.tensor_scalar(out=y[:], in0=y[:], scalar1=0.0,
                                        scalar2=None, op0=mybir.AluOpType.max)
                y2T = wp.tile([128, 2, 128], BF16, tag="x2w", bufs=2)
                for k in range(2):
                    psT = pp.tile([128, 128], BF16, space="PSUM", tag="pT")
                    nc.tensor.transpose(out=psT[:], in_=y[:, k * 128:(k + 1) * 128],
                                        identity=t_identb[:])
                    nc.scalar.copy(out=y2T[:, k, :], in_=psT[:])
                    nc.sync.dma_start(
                        out=d_x2oT[k * 128:(k + 1) * 128, w * 128:(w + 1) * 128],
                        in_=y2T[:, k, :])
            nc.gpsimd.collective_compute(
                "AllGather", mybir.AluOpType.bypass,
                replica_groups=[list(range(NCORES))],
                ins=[d_x2oT[:]], outs=[d_x2Tf[:]])

            # ================= layer 2 =================
            for t in dense_groups(0):
                t()
            for r in range(3):
                nxt = dense_groups(r + 1) if r < 2 else []
                edge_phase2(r, nxt)

            # ---- final: out = relu(x3 + b2) @ Wl + bl
            for w in range(W):
                y = wp.tile([128, HID], BF16, tag="t64")
                nc.vector.tensor_tensor(out=y[:], in0=x3acc[:, w * HID:(w + 1) * HID],
                                        in1=t_b2b[:], op=mybir.AluOpType.add)
                nc.vector.tensor_scalar(out=y[:], in0=y[:], scalar1=0.0,
                                        scalar2=None, op0=mybir.AluOpType.max)
                psT = pp.tile([128, 128], BF16, space="PSUM", tag="pT")
                nc.tensor.transpose(out=psT[:HID, :], in_=y[:], identity=t_identb[:])
                x3T = wp.tile([HID, 128], BF16, tag="x3T")
                nc.scalar.copy(out=x3T[:], in_=psT[:HID, :])
                pf = pp.tile([128, HID], F32, space="PSUM", tag="po")
                nc.tensor.matmul(out=pf[:], lhsT=x3T[:], rhs=t_wl[:],
                                 start=True, stop=True)
                fo = wp.tile([128, HID], F32, tag="fo")
                nc.vector.tensor_tensor(out=fo[:], in0=pf[:], in1=t_blb[:],
                                        op=mybir.AluOpType.add)
                nc.sync.dma_start(out=d_out[w * 128:(w + 1) * 128, :], in_=fo[:])

    nc.compile()
    return nc


# ------------------------------------------------------------------- driver

def kernel(x, e_blocks, e_spatial, e_similar, W1, a1s, a1d, b1,
           W2, a2s, a2d, b2, Wl, bl, **_ignored):
    global last_results
    x = np.asarray(x, np.float32)
    W1 = np.asarray(W1, np.float32)
    a1s = np.asarray(a1s, np.float32)
    a1d = np.asarray(a1d, np.float32)
    b1 = np.asarray(b1, np.float32)
    W2 = np.asarray(W2, np.float32)
    a2s = np.asarray(a2s, np.float32)
    a2d = np.asarray(a2d, np.float32)
    b2 = np.asarray(b2, np.float32)
    Wl = np.asarray(Wl, np.float32)
    bl = np.asarray(bl, np.float32)

    loops = np.arange(N, dtype=np.int64)
    edge_sets = []
    for ei, add_loops in ((e_blocks, False), (e_spatial, True), (e_similar, True)):
        src = np.asarray(ei[0], np.int64)
        dst = np.asarray(ei[1], np.int64)
        if add_loops:
            src = np.concatenate([src, loops])
            dst = np.concatenate([dst, loops])
        edge_sets.append((src, dst))

    win_of, core_of, slot_of = _assign_windows(edge_sets)

    xTb = _bf16(np.concatenate([x.T, np.zeros((IN_CH, NPAD - N), np.float32)],
                               axis=1))
    s1, cores1 = [], []
    s2, cores2 = [], []
    for src, dst in edge_sets:
        sch1, pc1 = _prep_edges_l1(src, dst, core_of, slot_of, xTb)
        s1.append(sch1)
        cores1.append(pc1)
        sch2, pc2 = _prep_edges_l2(src, dst, core_of, slot_of)
        s2.append(sch2)
        cores2.append(pc2)

    w1e = np.zeros((3, IN_CH, MMN), np.float32)
    w1ad = np.zeros((3, IN_CH, HEADS), np.float32)
    w2e = np.zeros((3, 2 * 128, MMN), np.float32)
    w2ad = np.zeros((3, 2 * 128, HEADS), np.float32)
    for r in range(3):
        w1e[r, :, :HC] = W1[r]
        w1e[r, :, HC:MMN] = W1[r] @ _block_diag_a(a1s[r])
        w1ad[r] = W1[r] @ _block_diag_a(a1d[r])
        w2e[r, :, :HC] = W2[r]
        w2e[r, :, HC:MMN] = W2[r] @ _block_diag_a(a2s[r])
        w2ad[r] = W2[r] @ _block_diag_a(a2d[r])
    w2e = w2e.reshape(3, 2, 128, MMN)
    w2ad = w2ad.reshape(3, 2, 128, HEADS)

    common = {
        "w1e": _bf16(w1e), "w1ad": _bf16(w1ad),
        "w2e": _bf16(w2e), "w2ad": _bf16(w2ad),
        "b1b": _bf16(np.tile(b1.sum(0)[None, :], (128, 1))),
        "b2b": _bf16(np.tile(b2.sum(0)[None, :], (128, 1))),
        "blb": np.tile(bl[None, :], (128, 1)).astype(np.float32),
        "wl": _bf16(Wl),
        "identb": _bf16(np.eye(128, dtype=np.float32)),
    }
    in_maps = []
    for c in range(NCORES):
        m = dict(common)
        for r in range(3):
            xs, xd, oh1 = cores1[r][c]
            m[f"xs{r}"] = xs
            m[f"xd{r}"] = xd
            m[f"oh1_{r}"] = oh1
            ia, oh2, ohT = cores2[r][c]
            m[f"idx{r}"] = ia
            m[f"oh2_{r}"] = oh2
            m[f"ohT{r}"] = ohT
        in_maps.append(m)

    nc = _build(s1, s2, win_of)
    res = run_bass_kernel_spmd(nc, in_maps, core_ids=list(range(NCORES)))
    last_results = res
    full = np.zeros((NPAD, HID), np.float32)
    for c in range(NCORES):
        oc = res.results[c]["out"]
        for s_ in range(W):
            w = int(win_of[c, s_])
            full[w * 128:(w + 1) * 128] = oc[s_ * 128:(s_ + 1) * 128]
    return full[:N].astype(np.float32)


# revision 31
# speedup vs baseline: 4.8302x; 1.0151x over previous
"""HeteroGAT (3-relation, 2-layer GAT + linear head) on 8 Trainium2 cores.

v3: Layer 1 is gather-free: the host pre-permutes x rows into per-edge
order (src and dst streams, f-major), streamed contiguously via HWDGE;
per-chunk projection matmuls produce h|al_s and accumulate al_d into the
same PSUM columns. Layer 2 keeps per-relation h2 tables in HBM with
dma_gather (768B rows), now 3-deep buffered. One-hots are built with
full-rate tensor_scalar is_equal ops (oh) and partition-broadcast
is_equal (ohT); exp(leaky(z)) = max(exp(z), exp(0.2z)) via two ACT ops.
Scatter is a single 260-wide matmul per 128-edge chunk with the softmax
denominators in columns 256:260. Only cross-core exchange: AllGather of
transposed layer-1 activations (bf16).
"""

import numpy as np

import concourse.bacc as bacc
import concourse.bass as bass
import concourse.mybir as mybir
import concourse.tile as tile
from concourse.bass_utils import run_bass_kernel_spmd

F32 = mybir.dt.float32
BF16 = mybir.dt.bfloat16
I16 = mybir.dt.int16

N = 50000
NPAD = 50176            # 392 * 128
NCORES = 8
NOWN = 6272             # 49 * 128 rows per core
W = 49                  # window slots per core
NT = NPAD // 128        # 392 global node tiles / windows
SPLIT = 32768           # int16 index limit for dma_gather
TABW = 384              # L2 table row stride in bf16 elems (768 B, %256)
HC = 256                # feature columns
MMN = 260               # 256 feats + 4 attention cols
IN_CH = 128
HID = 64
HEADS = 4
GMAX = 8                # chunks per dma_gather call (1024 idx is the ucode limit)

last_results = None


# ----------------------------------------------------------------- host prep

def _bf16(a):
    import ml_dtypes
    return np.asarray(a, np.float32).astype(ml_dtypes.bfloat16)


def _assign_windows(edge_sets):
    """Snake-assign the 392 global windows to (core, slot) by total edge
    count. Returns win_of [NCORES, W] and core_of/slot_of [NT]."""
    score = np.zeros(NT, np.int64)
    for src, dst in edge_sets:
        score += np.bincount(dst >> 7, minlength=NT)
    order = np.argsort(-score, kind="stable")
    win_of = np.zeros((NCORES, W), np.int64)
    for s in range(W):
        grp = order[s * NCORES:(s + 1) * NCORES]
        if s % 2:
            grp = grp[::-1]
        for c in range(NCORES):
            win_of[c, s] = grp[c]
    core_of = np.zeros(NT, np.int64)
    slot_of = np.zeros(NT, np.int64)
    for c in range(NCORES):
        for s in range(W):
            core_of[win_of[c, s]] = c
            slot_of[win_of[c, s]] = s
    return win_of, core_of, slot_of


def _prep_edges_l1(src, dst, core_of, slot_of, xTb):
    """Layer-1 host pre-gather. Edges bucketed by (core, slot); chunk counts
    maxed over cores so the device schedule is common. Returns
    (K1 [W], per_core list of (xs [128, totch*128] bf16,
    xd [...], dl [128, totch] bf16))."""
    E = src.size
    wg = dst >> 7
    sv = slot_of[wg]
    cv = core_of[wg]
    okey = cv * W + sv
    order = np.argsort(okey, kind="stable")
    so, ss, sd = okey[order], src[order], dst[order]
    cnts = np.bincount(okey, minlength=NCORES * W)
    seg_start = np.concatenate([[0], np.cumsum(cnts)[:-1]])
    pos = np.arange(E) - seg_start[so]

    mx = cnts.reshape(NCORES, W).max(axis=0)
    K1 = np.maximum((mx + 127) // 128, 1)
    chbase = np.concatenate([[0], np.cumsum(K1)[:-1]])
    totch = int(K1.sum())

    svs, cvs = sv[order], cv[order]
    col = (chbase[svs] + (pos >> 7)) * 128 + (pos & 127)

    eye_ext = np.zeros((129, 128), np.float32)
    eye_ext[1:] = np.eye(128, dtype=np.float32)
    eye_ext = _bf16(eye_ext)
    per_core = []
    for c in range(NCORES):
        m = cvs == c
        srccols = np.zeros(totch * 128, np.int64)
        dstcols = np.zeros(totch * 128, np.int64)
        dlv = np.full(totch * 128, -1, np.int64)
        srccols[col[m]] = ss[m]
        dstcols[col[m]] = sd[m]
        dlv[col[m]] = sd[m] & 127
        xs = np.ascontiguousarray(xTb[:, srccols])
        xd = np.ascontiguousarray(xTb[:, dstcols])
        oh_rows = eye_ext[dlv + 1].reshape(totch, 128, 128)
        oh = np.ascontiguousarray(
            oh_rows.transpose(1, 0, 2).reshape(128, totch * 128))
        per_core.append((xs, xd, oh))
    return dict(K1=K1, chbase=chbase, totch=totch), per_core


def _prep_edges_l2(src, dst, core_of, slot_of):
    """Layer-2 gather schedule (same as baseline) + dlT row for ohT builds.
    Returns (sched, per_core list of (idx [128, totcol] i16,
    dl [128, totch] bf16, dlT [1, totch*128] bf16))."""
    E = src.size
    wg = dst >> 7
    sv = slot_of[wg]
    cv = core_of[wg]
    # gather table rows live in (core, slot) order
    swg = src >> 7
    srcpos = (core_of[swg] * W + slot_of[swg]) * 128 + (src & 127)
    gv = (srcpos >= SPLIT).astype(np.int64)
    okey = cv * (2 * W) + sv * 2 + gv
    order = np.argsort(okey, kind="stable")
    so, ss, sd = okey[order], src[order], dst[order]
    cnts = np.bincount(okey, minlength=NCORES * 2 * W)
    seg_start = np.concatenate([[0], np.cumsum(cnts)[:-1]])
    pos = np.arange(E) - seg_start[so]

    mx = cnts.reshape(NCORES, 2 * W).max(axis=0)
    Kg = (mx + 127) // 128
    KL, KH = Kg[0::2].copy(), Kg[1::2].copy()
    KL[(KL + KH) == 0] = 1
    Ksum = KL + KH
    chbase = np.concatenate([[0], np.cumsum(Ksum)[:-1]])
    totch = int(Ksum.sum())
    colL = np.zeros(W, np.int64)
    colH = np.zeros(W, np.int64)
    cum = 0
    for w in range(W):
        colL[w] = cum
        cum += KL[w] * 8
        colH[w] = cum
        cum += KH[w] * 8
    totcol = int(cum)

    svs, gvs, cvs = sv[order], gv[order], cv[order]
    j = np.where(gvs == 0, pos, KL[svs] * 128 + pos)
    ch = chbase[svs] + (j >> 7)
    prow = j & 127
    colbase = np.where(gvs == 0, colL[svs], colH[svs])
    icol = colbase + (pos >> 4)
    irow = pos & 15
    sp_s = srcpos[order]
    idxval = np.where(gvs == 0, sp_s, sp_s - SPLIT).astype(np.int16)
    dloc = (sd & 127).astype(np.float32)

    eye_ext = np.zeros((129, 128), np.float32)
    eye_ext[1:] = np.eye(128, dtype=np.float32)
    eye_ext = _bf16(eye_ext)
    per_core = []
    for c in range(NCORES):
        m = cvs == c
        ia = np.zeros((16, totcol), np.int16)
        ia[irow[m], icol[m]] = idxval[m]
        dlv = np.full(totch * 128, -1, np.int64)
        dlv[ch[m] * 128 + prow[m]] = sd[m] & 127
        oh_rows = eye_ext[dlv + 1].reshape(totch, 128, 128)
        oh = np.ascontiguousarray(
            oh_rows.transpose(1, 0, 2).reshape(128, totch * 128))
        ohT = np.ascontiguousarray(
            oh_rows.transpose(2, 0, 1).reshape(128, totch * 128))
        per_core.append((np.ascontiguousarray(np.tile(ia, (8, 1))),
                         oh, ohT))
    sched = dict(KL=KL, KH=KH, chbase=chbase, colL=colL, colH=colH,
                 totch=totch, totcol=totcol)
    return sched, per_core


def _block_diag_a(a):            # a: [4, 64] -> [256, 4]
    A = np.zeros((HEADS * HID, HEADS), np.float32)
    A[np.arange(HEADS * HID), np.arange(HEADS * HID) // HID] = a.reshape(-1)
    return A


# ------------------------------------------------------------ device program

def _build(s1, s2, win_of):
    nc = bacc.Bacc("TRN2", num_devices=NCORES, num_swdge_queues=4)
    Exp = mybir.ActivationFunctionType.Exp

    d_xs, d_xd, d_oh1 = [], [], []
    for r in range(3):
        t1 = s1[r]["totch"]
        d_xs.append(nc.dram_tensor(f"xs{r}", [128, t1 * 128], BF16,
                                   kind="ExternalInput"))
        d_xd.append(nc.dram_tensor(f"xd{r}", [128, t1 * 128], BF16,
                                   kind="ExternalInput"))
        d_oh1.append(nc.dram_tensor(f"oh1_{r}", [128, t1 * 128], BF16,
                                    kind="ExternalInput"))
    d_idx, d_oh2, d_ohT = [], [], []
    for r in range(3):
        s = s2[r]
        d_idx.append(nc.dram_tensor(f"idx{r}", [128, s["totcol"]], I16,
                                    kind="ExternalInput"))
        d_oh2.append(nc.dram_tensor(f"oh2_{r}", [128, s["totch"] * 128], BF16,
                                    kind="ExternalInput"))
        d_ohT.append(nc.dram_tensor(f"ohT{r}", [128, s["totch"] * 128], BF16,
                                    kind="ExternalInput"))
    d_w1e = nc.dram_tensor("w1e", [3, IN_CH, MMN], BF16, kind="ExternalInput")
    d_w1ad = nc.dram_tensor("w1ad", [3, IN_CH, HEADS], BF16, kind="ExternalInput")
    d_w2e = nc.dram_tensor("w2e", [3, 2, 128, MMN], BF16, kind="ExternalInput")
    d_w2ad = nc.dram_tensor("w2ad", [3, 2, 128, HEADS], BF16, kind="ExternalInput")
    d_b1b = nc.dram_tensor("b1b", [128, HC], BF16, kind="ExternalInput")
    d_b2b = nc.dram_tensor("b2b", [128, HID], BF16, kind="ExternalInput")
    d_blb = nc.dram_tensor("blb", [128, HID], F32, kind="ExternalInput")
    d_wl = nc.dram_tensor("wl", [HID, HID], BF16, kind="ExternalInput")
    d_identb = nc.dram_tensor("identb", [128, 128], BF16, kind="ExternalInput")

    tab2 = [nc.dram_tensor(f"tab2_{r}", [NPAD, TABW], BF16, kind="Internal")
            for r in range(3)]
    d_x2oT = nc.dram_tensor("x2oT", [2 * 128, NOWN], BF16, kind="Internal")
    d_x2Tf = nc.dram_tensor("x2Tf", [NCORES * 2 * 128, NOWN], BF16,
                            kind="Internal", addr_space="Shared")
    d_out = nc.dram_tensor("out", [NOWN, HID], F32, kind="ExternalOutput")

    qn = [0]    # rotating SWDGE queue

    with tile.TileContext(nc) as tc:
        with (
            tc.tile_pool(name="const", bufs=1) as cp,
            tc.tile_pool(name="acc", bufs=1) as ac,
            tc.tile_pool(name="acc2", bufs=1) as ac2,
            tc.tile_pool(name="eio", bufs=2) as ei,
            tc.tile_pool(name="gw", bufs=2) as gp,
            tc.tile_pool(name="st", bufs=2) as sp,
            tc.tile_pool(name="work", bufs=2) as wp,
            tc.tile_pool(name="ps", bufs=2, space="PSUM") as pp,
        ):
            # ---- constants
            t_identb = cp.tile([128, 128], BF16)
            nc.sync.dma_start(out=t_identb[:], in_=d_identb[:])
            t_w1e = [cp.tile([IN_CH, MMN], BF16, tag=f"w1e{r}", name=f"w1e{r}")
                     for r in range(3)]
            t_w1ad = [cp.tile([IN_CH, HEADS], BF16, tag=f"w1ad{r}", name=f"w1ad{r}")
                      for r in range(3)]
            for r in range(3):
                nc.sync.dma_start(out=t_w1e[r][:], in_=d_w1e[r])
                nc.sync.dma_start(out=t_w1ad[r][:], in_=d_w1ad[r])
            t_w2e = [[cp.tile([128, MMN], BF16, tag=f"w2e{r}{k}", name=f"w2e{r}{k}")
                      for k in range(2)] for r in range(3)]
            t_w2ad = [[cp.tile([128, HEADS], BF16, tag=f"w2ad{r}{k}",
                               name=f"w2ad{r}{k}") for k in range(2)]
                      for r in range(3)]
            for r in range(3):
                for k in range(2):
                    nc.sync.dma_start(out=t_w2e[r][k][:], in_=d_w2e[r, k])
                    nc.sync.dma_start(out=t_w2ad[r][k][:], in_=d_w2ad[r, k])
            t_b1b = cp.tile([128, HC], BF16)
            nc.sync.dma_start(out=t_b1b[:], in_=d_b1b[:])
            t_b2b = cp.tile([128, HID], BF16)
            nc.sync.dma_start(out=t_b2b[:], in_=d_b2b[:])
            t_blb = cp.tile([128, HID], F32)
            nc.sync.dma_start(out=t_blb[:], in_=d_blb[:])
            t_wl = cp.tile([HID, HID], BF16)
            nc.sync.dma_start(out=t_wl[:], in_=d_wl[:])
            x3acc = ac.tile([128, W * HID], BF16)

            K1max = max(int(s1[r]["K1"].max()) for r in range(3))
            K2max = max(int((s2[r]["KL"] + s2[r]["KH"]).max()) for r in range(3))

            # ---------------- layer-1 edge phase: stream + project ----------
            # (window-major, relation-inner; x2 finalized per window)
            x2win = {}

            def l1_stage_a(w, r):
                K = int(s1[r]["K1"][w])
                c0 = int(s1[r]["chbase"][w])
                xs = sp.tile([128, K1max * 128], BF16, tag="xs", name="xs")
                nc.sync.dma_start(
                    out=xs[:, 0:K * 128],
                    in_=d_xs[r][:, c0 * 128:(c0 + K) * 128])
                xd = sp.tile([128, K1max * 128], BF16, tag="xd", name="xd")
                nc.sync.dma_start(
                    out=xd[:, 0:K * 128],
                    in_=d_xd[r][:, c0 * 128:(c0 + K) * 128])
                hall = wp.tile([128, K1max, MMN], BF16, tag="hall", bufs=3)
                for c in range(K):
                    ph = pp.tile([128, MMN], F32, space="PSUM", tag="ph")
                    nc.tensor.matmul(out=ph[:],
                                     lhsT=xs[:, c * 128:(c + 1) * 128],
                                     rhs=t_w1e[r][:], start=True, stop=False,
                                     skip_group_check=True)
                    nc.tensor.matmul(out=ph[:, HC:MMN],
                                     lhsT=xd[:, c * 128:(c + 1) * 128],
                                     rhs=t_w1ad[r][:], start=False, stop=True,
                                     skip_group_check=True)
                    nc.scalar.copy(out=hall[:, c, :], in_=ph[:])
                return (w, r, K, c0, hall)

            def l1_stage_b(st):
                w, r, K, c0, hall = st
                oh1 = sp.tile([128, K1max, 128], BF16, tag="oh1", name="oh1")
                nc.sync.dma_start(
                    out=oh1[:, 0:K, :].rearrange("p k d -> p (k d)"),
                    in_=d_oh1[r][:, c0 * 128:(c0 + K) * 128])
                zl = wp.tile([128, K1max, HEADS], BF16, tag="zl", bufs=2)
                nc.vector.scalar_tensor_tensor(
                    out=zl[:, 0:K, :], in0=hall[:, 0:K, HC:MMN], scalar=0.2,
                    in1=hall[:, 0:K, HC:MMN],
                    op0=mybir.AluOpType.mult, op1=mybir.AluOpType.max)
                nc.scalar.activation(out=hall[:, 0:K, HC:MMN],
                                     in_=zl[:, 0:K, :], func=Exp)
                for h in range(HEADS):
                    nc.vector.tensor_tensor(
                        out=hall[:, 0:K, h * HID:(h + 1) * HID],
                        in0=hall[:, 0:K, h * HID:(h + 1) * HID],
                        in1=hall[:, 0:K, HC + h:HC + h + 1]
                            .to_broadcast([128, K, HID]),
                        op=mybir.AluOpType.mult)
                return (w, r, K, oh1, hall)

            def l1_stage_c(st):
                w, r, K, oh1, hall = st
                po = pp.tile([128, MMN], F32, space="PSUM", tag="po")
                for c in range(K):
                    nc.tensor.matmul(out=po[:], lhsT=oh1[:, c, :],
                                     rhs=hall[:, c, :],
                                     start=(c == 0), stop=(c == K - 1))
                _epilogue1(r, w, po)
                if r == 2:
                    _x2_finalize(w)

            def _x2_finalize(w):
                acc = x2win.pop(w)
                y = wp.tile([128, HC], BF16, tag="y")
                nc.vector.tensor_tensor(out=y[:], in0=acc[:],
                                        in1=t_b1b[:], op=mybir.AluOpType.add)
                nc.vector.tensor_scalar(out=y[:], in0=y[:], scalar1=0.0,
                                        scalar2=None, op0=mybir.AluOpType.max)
                y2T = wp.tile([128, 2, 128], BF16, tag="x2w", bufs=2)
                for k in range(2):
                    psT = pp.tile([128, 128], BF16, space="PSUM", tag="pT", bufs=1)
                    nc.tensor.transpose(out=psT[:], in_=y[:, k * 128:(k + 1) * 128],
                                        identity=t_identb[:])
                    nc.scalar.copy(out=y2T[:, k, :], in_=psT[:])
                nc.sync.dma_start(
                    out=d_x2oT[:, w * 128:(w + 1) * 128]
                        .rearrange("(k p) n -> p k n", p=128),
                    in_=y2T[:])

            def _epilogue1(r, w, po):
                pon = wp.tile([128, HC], BF16, tag="pon", bufs=3)
                nc.vector.tensor_copy(out=pon[:], in_=po[:, 0:HC])
                rd = wp.tile([128, HEADS], F32, tag="rd", bufs=3)
                nc.vector.tensor_scalar(
                    out=rd[:], in0=po[:, HC:MMN], scalar1=1e-16,
                    scalar2=None, op0=mybir.AluOpType.add)
                nc.vector.reciprocal(out=rd[:], in_=rd[:])
                if r == 0:
                    x2win[w] = ac2.tile([128, HC], BF16, tag=f"x2a{w % 4}",
                                        name=f"x2a{w % 4}")
                dstap = x2win[w][:]
                if r == 0:
                    for h in range(HEADS):
                        nc.vector.tensor_scalar(
                            out=dstap[:, h * HID:(h + 1) * HID],
                            in0=pon[:, h * HID:(h + 1) * HID],
                            scalar1=rd[:, h:h + 1], scalar2=None,
                            op0=mybir.AluOpType.mult)
                else:
                    tmp = wp.tile([128, HC], BF16, tag="tmp")
                    for h in range(HEADS):
                        nc.scalar.mul(
                            out=tmp[:, h * HID:(h + 1) * HID],
                            in_=pon[:, h * HID:(h + 1) * HID],
                            mul=rd[:, h:h + 1])
                    nc.vector.tensor_tensor(
                        out=dstap, in0=dstap, in1=tmp[:],
                        op=mybir.AluOpType.add)

            def layer1():
                seq = [(w, r) for w in range(W) for r in range(3)]
                states = {}
                for i, (w, r) in enumerate(seq):
                    states[i] = l1_stage_a(w, r)
                    if i >= 1:
                        states[i - 1] = l1_stage_b(states[i - 1])
                    if i >= 2:
                        l1_stage_c(states.pop(i - 2))
                n = len(seq)
                states[n - 1] = l1_stage_b(states[n - 1])
                l1_stage_c(states.pop(n - 2))
                l1_stage_c(states.pop(n - 1))

            # ---------------- layer-2 dense phase: h2 tables ----------------
            # table rows are in (core, slot) order so each 4-slot group is one
            # contiguous write
            def dense_groups(r):
                DB = 4
                thunks = []
                for co in range(NCORES):
                    for so0 in range(0, W, DB):
                        nb = min(DB, W - so0)

                        def run(co=co, so0=so0, nb=nb):
                            lhs4 = []
                            for k in range(2):
                                l4 = wp.tile([128, DB * 128], BF16,
                                             tag=f"lhs4{k}", bufs=2, name=f"l4{k}")
                                nc.sync.dma_start(
                                    out=l4[:, 0:nb * 128],
                                    in_=d_x2Tf[co * 256 + k * 128:
                                               co * 256 + (k + 1) * 128,
                                               so0 * 128:(so0 + nb) * 128])
                                lhs4.append(l4)
                            hsb4 = wp.tile([128, DB, MMN], BF16, tag="hsbd",
                                           bufs=2)
                            for i in range(nb):
                                ph = pp.tile([128, MMN], F32, space="PSUM",
                                             tag="ph")
                                for k in range(2):
                                    nc.tensor.matmul(
                                        out=ph[:],
                                        lhsT=lhs4[k][:, i * 128:(i + 1) * 128],
                                        rhs=t_w2e[r][k][:],
                                        start=(k == 0), stop=(k == 1))
                                nc.scalar.copy(out=hsb4[:, i, :], in_=ph[:])
                            base = (co * W + so0) * 128
                            nc.sync.dma_start(
                                out=tab2[r][base:base + nb * 128, 0:MMN]
                                    .rearrange("(k p) c -> p k c", p=128),
                                in_=hsb4[:, 0:nb, :])

                        thunks.append(run)
                return thunks

            # ---------------- layer-2 edge phase: gather ------------------
            def edge_phase2(r, dense_next):
                s = s2[r]
                KL, KH = s["KL"], s["KH"]
                chb, colL, colH = s["chbase"], s["colL"], s["colH"]
                t_idx = ei.tile([128, s["totcol"]], I16, tag="idx")
                nc.sync.dma_start(out=t_idx[:], in_=d_idx[r][:])

                def stage_a(w):
                    kl, kh = int(KL[w]), int(KH[w])
                    K = kl + kh
                    c0 = int(chb[w])
                    g = gp.tile([128, K2max, TABW], BF16, tag="gw", bufs=4)
                    for grp, (kk, coff, base) in enumerate(
                            ((kl, int(colL[w]), 0), (kh, int(colH[w]), kl))):
                        src_ap = tab2[r][:] if grp == 0 else tab2[r][SPLIT:NPAD, :]
                        for sub in range(0, kk, GMAX):
                            nk = min(GMAX, kk - sub)
                            nc.gpsimd.dma_gather(
                                g[:, base + sub:base + sub + nk, :], src_ap,
                                t_idx[:, coff + sub * 8:coff + (sub + nk) * 8],
                                nk * 128, nk * 128, TABW,
                                queue_num=qn[0] % 4)
                            qn[0] += 1
                    ohT = gp.tile([128, K2max, 128], BF16, tag="ohT",
                                  bufs=2, name="ohT")
                    nc.sync.dma_start(
                        out=ohT[:, 0:K, :].rearrange("p k d -> p (k d)"),
                        in_=d_ohT[r][:, c0 * 128:(c0 + K) * 128])
                    # al_d for this window's own 128 dst rows
                    x2w = wp.tile([128, 2, 128], BF16, tag="x2w", bufs=2)
                    nc.sync.dma_start(
                        out=x2w[:],
                        in_=d_x2oT[:, w * 128:(w + 1) * 128]
                            .rearrange("(k p) n -> p k n", p=128))
                    paw = pp.tile([128, HEADS], F32, space="PSUM", tag="pT", bufs=1)
                    for k in range(2):
                        nc.tensor.matmul(
                            out=paw[:], lhsT=x2w[:, k, :],
                            rhs=t_w2ad[r][k][:],
                            start=(k == 0), stop=(k == 1))
                    aw = wp.tile([128, HEADS], BF16, tag="aw", bufs=2)
                    nc.vector.tensor_copy(out=aw[:], in_=paw[:])
                    return (w, K, c0, g, ohT, aw)

                def stage_b(st):
                    w, K, c0, g, ohT, aw = st
                    t_oh = gp.tile([128, K2max, 128], BF16, tag="ohL",
                                   bufs=2, name="ohL")
                    nc.sync.dma_start(
                        out=t_oh[:, 0:K, :].rearrange("p k d -> p (k d)"),
                        in_=d_oh2[r][:, c0 * 128:(c0 + K) * 128])
                    pz = pp.tile([128, K2max, HEADS], F32, space="PSUM",
                                 tag="pald", bufs=1)
                    for c in range(K):
                        nc.tensor.matmul(out=pz[:, c, :], lhsT=ohT[:, c, :],
                                         rhs=aw[:], start=True, stop=True,
                                         skip_group_check=True)
                    zl = wp.tile([128, K2max, HEADS], BF16, tag="zl", bufs=2)
                    nc.vector.tensor_tensor(
                        out=zl[:, 0:K, :], in0=g[:, 0:K, HC:MMN],
                        in1=pz[:, 0:K, :], op=mybir.AluOpType.add)
                    nc.vector.scalar_tensor_tensor(
                        out=zl[:, 0:K, :], in0=zl[:, 0:K, :], scalar=0.2,
                        in1=zl[:, 0:K, :],
                        op0=mybir.AluOpType.mult, op1=mybir.AluOpType.max)
                    nc.scalar.activation(out=g[:, 0:K, HC:MMN],
                                         in_=zl[:, 0:K, :], func=Exp)
                    for h in range(HEADS):
                        nc.vector.tensor_tensor(
                            out=g[:, 0:K, h * HID:(h + 1) * HID],
                            in0=g[:, 0:K, h * HID:(h + 1) * HID],
                            in1=g[:, 0:K, HC + h:HC + h + 1]
                                .to_broadcast([128, K, HID]),
                            op=mybir.AluOpType.mult)
                    return (w, K, t_oh, g)

                def stage_c(st):
                    w, K, t_oh, g = st
                    po = pp.tile([128, MMN], F32, space="PSUM", tag="po")
                    for c in range(K):
                        nc.tensor.matmul(out=po[:], lhsT=t_oh[:, c, :],
                                         rhs=g[:, c, 0:MMN],
                                         start=(c == 0), stop=(c == K - 1))
                    _epilogue2(r, w, po)

                nd = len(dense_next)
                emitted = 0
                states = {}
                for w in range(W):
                    states[w] = stage_a(w)
                    want = (w + 1) * nd // W
                    while emitted < want:
                        dense_next[emitted]()
                        emitted += 1
                    if w >= 1:
                        states[w - 1] = stage_b(states[w - 1])
                    if w >= 2:
                        stage_c(states.pop(w - 2))
                states[W - 1] = stage_b(states[W - 1])
                stage_c(states.pop(W - 2))
                stage_c(states.pop(W - 1))

            def _epilogue2(r, w, po):
                pon = wp.tile([128, HC], BF16, tag="pon", bufs=3)
                nc.vector.tensor_copy(out=pon[:], in_=po[:, 0:HC])
                rd = wp.tile([128, HEADS], F32, tag="rd", bufs=3)
                nc.vector.tensor_scalar(
                    out=rd[:], in0=po[:, HC:MMN], scalar1=1e-16,
                    scalar2=None, op0=mybir.AluOpType.add)
                nc.vector.reciprocal(out=rd[:], in_=rd[:])
                nc.vector.tensor_scalar(
                    out=rd[:], in0=rd[:], scalar1=0.25, scalar2=None,
                    op0=mybir.AluOpType.mult)
                dstap = x3acc[:, w * HID:(w + 1) * HID]
                for h in range(HEADS):
                    if r == 0 and h == 0:
                        nc.vector.tensor_scalar(
                            out=dstap, in0=pon[:, 0:HID],
                            scalar1=rd[:, 0:1], scalar2=None,
                            op0=mybir.AluOpType.mult)
                    else:
                        t64 = wp.tile([128, HID], BF16, tag="t64")
                        nc.scalar.mul(out=t64[:], in_=pon[:, h * HID:(h + 1) * HID],
                                      mul=rd[:, h:h + 1])
                        nc.vector.tensor_tensor(
                            out=dstap, in0=dstap, in1=t64[:],
                            op=mybir.AluOpType.add)

            # ================= layer 1 =================
            layer1()

            nc.gpsimd.collective_compute(
                "AllGather", mybir.AluOpType.bypass,
                replica_groups=[list(range(NCORES))],
                ins=[d_x2oT[:]], outs=[d_x2Tf[:]])

            # ================= layer 2 =================
            for t in dense_groups(0):
                t()
            for r in range(3):
                nxt = dense_groups(r + 1) if r < 2 else []
                edge_phase2(r, nxt)

            # ---- final: out = relu(x3 + b2) @ Wl + bl
            for w in range(W):
                y = wp.tile([128, HID], BF16, tag="t64")
                nc.vector.tensor_tensor(out=y[:], in0=x3acc[:, w * HID:(w + 1) * HID],
                                        in1=t_b2b[:], op=mybir.AluOpType.add)
                nc.vector.tensor_scalar(out=y[:], in0=y[:], scalar1=0.0,
                                        scalar2=None, op0=mybir.AluOpType.max)
                psT = pp.tile([128, 128], BF16, space="PSUM", tag="pT", bufs=1)
                nc.tensor.transpose(out=psT[:HID, :], in_=y[:], identity=t_identb[:])
                x3T = wp.tile([HID, 128], BF16, tag="x3T")
                nc.scalar.copy(out=x3T[:], in_=psT[:HID, :])
                pf = pp.tile([128, HID], F32, space="PSUM", tag="po")
                nc.tensor.matmul(out=pf[:], lhsT=x3T[:], rhs=t_wl[:],
                                 start=True, stop=True)
                fo = wp.tile([128, HID], F32, tag="fo")
                nc.vector.tensor_tensor(out=fo[:], in0=pf[:], in1=t_blb[:],
                                        op=mybir.AluOpType.add)
                nc.sync.dma_start(out=d_out[w * 128:(w + 1) * 128, :], in_=fo[:])

    nc.compile()
    return nc


# ------------------------------------------------------------------- driver

def kernel(x, e_blocks, e_spatial, e_similar, W1, a1s, a1d, b1,
           W2, a2s, a2d, b2, Wl, bl, **_ignored):
    global last_results
    x = np.asarray(x, np.float32)
    W1 = np.asarray(W1, np.float32)
    a1s = np.asarray(a1s, np.float32)
    a1d = np.asarray(a1d, np.float32)
    b1 = np.asarray(b1, np.float32)
    W2 = np.asarray(W2, np.float32)
    a2s = np.asarray(a2s, np.float32)
    a2d = np.asarray(a2d, np.float32)
    b2 = np.asarray(b2, np.float32)
    Wl = np.asarray(Wl, np.float32)
    bl = np.asarray(bl, np.float32)

    loops = np.arange(N, dtype=np.int64)
    edge_sets = []
    for ei, add_loops in ((e_blocks, False), (e_spatial, True), (e_similar, True)):
        src = np.asarray(ei[0], np.int64)
        dst = np.asarray(ei[1], np.int64)
        if add_loops:
            src = np.concatenate([src, loops])
            dst = np.concatenate([dst, loops])
        edge_sets.append((src, dst))

    win_of, core_of, slot_of = _assign_windows(edge_sets)

    xTb = _bf16(np.concatenate([x.T, np.zeros((IN_CH, NPAD - N), np.float32)],
                               axis=1))
    s1, cores1 = [], []
    s2, cores2 = [], []
    for src, dst in edge_sets:
        sch1, pc1 = _prep_edges_l1(src, dst, core_of, slot_of, xTb)
        s1.append(sch1)
        cores1.append(pc1)
        sch2, pc2 = _prep_edges_l2(src, dst, core_of, slot_of)
        s2.append(sch2)
        cores2.append(pc2)

    w1e = np.zeros((3, IN_CH, MMN), np.float32)
    w1ad = np.zeros((3, IN_CH, HEADS), np.float32)
    w2e = np.zeros((3, 2 * 128, MMN), np.float32)
    w2ad = np.zeros((3, 2 * 128, HEADS), np.float32)
    for r in range(3):
        w1e[r, :, :HC] = W1[r]
        w1e[r, :, HC:MMN] = W1[r] @ _block_diag_a(a1s[r])
        w1ad[r] = W1[r] @ _block_diag_a(a1d[r])
        w2e[r, :, :HC] = W2[r]
        w2e[r, :, HC:MMN] = W2[r] @ _block_diag_a(a2s[r])
        w2ad[r] = W2[r] @ _block_diag_a(a2d[r])
    w2e = w2e.reshape(3, 2, 128, MMN)
    w2ad = w2ad.reshape(3, 2, 128, HEADS)

    common = {
        "w1e": _bf16(w1e), "w1ad": _bf16(w1ad),
        "w2e": _bf16(w2e), "w2ad": _bf16(w2ad),
        "b1b": _bf16(np.tile(b1.sum(0)[None, :], (128, 1))),
        "b2b": _bf16(np.tile(b2.sum(0)[None, :], (128, 1))),
        "blb": np.tile(bl[None, :], (128, 1)).astype(np.float32),
        "wl": _bf16(Wl),
        "identb": _bf16(np.eye(128, dtype=np.float32)),
    }
    in_maps = []
    for c in range(NCORES):
        m = dict(common)
        for r in range(3):
            xs, xd, oh1 = cores1[r][c]
            m[f"xs{r}"] = xs
            m[f"xd{r}"] = xd
            m[f"oh1_{r}"] = oh1
            ia, oh2, ohT = cores2[r][c]
            m[f"idx{r}"] = ia
            m[f"oh2_{r}"] = oh2
            m[f"ohT{r}"] = ohT
        in_maps.append(m)

    nc = _build(s1, s2, win_of)
    res = run_bass_kernel_spmd(nc, in_maps, core_ids=list(range(NCORES)))
    last_results = res
    full = np.zeros((NPAD, HID), np.float32)
    for c in range(NCORES):
        oc = res.results[c]["out"]
        for s_ in range(W):
            w = int(win_of[c, s_])
            full[w * 128:(w + 1) * 128] = oc[s_ * 128:(s_ + 1) * 128]
    return full[:N].astype(np.float32)
